# revision 30
# baseline (speedup 1.0000x reference)
"""GAT 2-layer encoder on 8 Trainium2 NeuronCores.

Reference computation: layer 1 = GAT conv over edge_index[:, :500] (weights W1),
layer 2 = GAT conv over edge_index[:, 500:] (weights W2).

Strategy (sparse-special):
  - Layer-1 output x1 differs from b1 only on the K<=500 distinct dsts of the
    first 500 edges ("specials").  In layer 2, h2[src] = x1[src]@W2 is the
    constant default row for every non-special src, so only edges whose src is
    special (~8k of 1.6M) carry information.  For a dst with no special
    in-edge, softmax over equal scores gives alpha = 1/deg for every in-edge,
    hence out = b1@W2 + b2 exactly (up to the 1e-16 eps), a CONSTANT row.
  - Device builds a (K+2)-row table in h2-space with b2 baked in:
    row r = [x1_r@W2 + b2 | asrc2_r | adst2_r], row 0 = default, row K+1 = b2
    (for deg-0 dsts), rows K+2.. = replicas of row 0 (spread gather load).
  - Sharding: dst-range partition of the 1.6M layer-2 edges across 8 cores (no
    collectives; layer 1 + table build replicated on every core, it is tiny).
  - Per core: dsts sorted so that special-adst / deg-0 dsts land in block 0
    (full slot grid: special slots + default slot + dst slot, all gathered),
    remaining computed blocks carry ONLY special-edge slots; their default
    in-edge mass and adst2 = c_d are handled with per-position scalars.  The
    ~90 all-default blocks are written with one broadcast DMA of the constant
    row.  Layer-2 table rows are pre-transformed by W2, so no matmul there.
"""

import sys

sys.path.insert(0, "/opt/trn_rl_repo")

from contextlib import ExitStack

import numpy as np

import concourse.bacc as bacc
import concourse.bass as bass
import concourse.mybir as mybir
import concourse.tile as tile
from concourse.bass_utils import run_bass_kernel_spmd
from concourse.masks import make_identity

F32 = mybir.dt.float32
I16 = mybir.dt.int16
I32 = mybir.dt.int32
AF = mybir.ActivationFunctionType
OP = mybir.AluOpType

N = 100000
D = 64
NCORES = 8
NPC = N // NCORES          # dst nodes per core
P = 128
NSPLIT = 500               # first 500 edges -> layer 1
NEG_SLOPE = 0.2
BIG = 200.0                # score shift so padded slots underflow exp to 0.0
VTAB = 1024                # gather table rows (specials + default replicas)
NREPL = 512                # default-row replicas written (one broadcast DMA)


def _wrap16(flat):
    """int16 stream [n] (n%16==0) -> dma_gather idx tile [128, n//16]."""
    w = flat.reshape(-1, 16).T
    return np.ascontiguousarray(np.tile(w, (8, 1)).astype(np.int16))


def _groups_of(Ls, b0=0):
    """Split the per-block padded-degree profile into equal-L runs."""
    groups = []
    off = 0
    b = 0
    while b < len(Ls):
        s = b
        while b < len(Ls) and Ls[b] == Ls[s]:
            b += 1
        groups.append({"b0": b0 + s, "B": b - s, "L": Ls[s], "slot_off": off})
        off += (b - s) * Ls[s]
    return groups


def prep(inputs):
    """Host-side index prep (pure index computation, no feature values)."""
    ei = np.asarray(inputs["edge_index"])
    src = ei[0].astype(np.int64)
    dst = ei[1].astype(np.int64)
    s1, d1 = src[:NSPLIT], dst[:NSPLIT]
    s2, d2 = src[NSPLIT:], dst[NSPLIT:]

    # ---- layer 1 structure ----
    specials, deg1 = np.unique(d1, return_counts=True)
    K = len(specials)
    order1 = np.argsort(-deg1, kind="stable")
    spec_by_pos = specials[order1]          # grid position q -> node, table row q+1
    rowmap = np.zeros(N, np.int16)
    rowmap[spec_by_pos] = np.arange(1, K + 1)
    nblk1 = (K + P - 1) // P
    npos1 = nblk1 * P

    U = np.unique(np.concatenate([s1, d1]))
    nU = len(U)
    # x-row gather in int16-addressable ranges of 32768 rows
    RSPAN = 1 << 15
    xranges = []          # (lo, ntiles)
    uidx16_parts = []
    uindex = np.zeros(N, np.int64)
    off = 0
    for lo in range(0, N, RSPAN):
        hi = min(lo + RSPAN, N)
        Ur = U[(U >= lo) & (U < hi)]
        if len(Ur) == 0:
            continue
        nt = (len(Ur) + P - 1) // P
        pad = np.full(nt * P, lo, np.int64)
        pad[:len(Ur)] = Ur
        uindex[Ur] = off * P + np.arange(len(Ur))
        uidx16_parts.append(_wrap16((pad - lo).astype(np.int16)))
        xranges.append((lo, nt))
        off += nt
    nUt = off
    uidx16 = np.concatenate(uidx16_parts, axis=1)

    # layer-1 slot grid: per block [special-edge slots | dst slot]
    rank1 = np.empty(K, np.int64)
    rank1[order1] = np.arange(K)
    d1pos = rank1[np.searchsorted(specials, d1)]
    deg1_sorted = np.zeros(npos1, np.int64)
    deg1_sorted[:K] = deg1[order1]
    L1sp = [max(int(deg1_sorted[b * P:(b + 1) * P].max()), 1)
            for b in range(nblk1)]
    L1 = [l + 1 for l in L1sp]
    S1 = int(sum(L1))
    slot_base1 = np.concatenate([[0], np.cumsum(L1)])[:-1]
    idx1 = np.zeros(S1 * P, np.int16)
    mask1 = np.zeros(S1 * P, np.float32)
    pe = np.argsort(d1pos, kind="stable")
    pos_s = d1pos[pe]
    val_s = uindex[s1[pe]].astype(np.int16)
    start_of_pos = np.searchsorted(pos_s, np.arange(npos1))
    kk = np.arange(len(pos_s)) - start_of_pos[pos_s]
    flat = (slot_base1[pos_s // P] + kk) * P + (pos_s % P)
    idx1[flat] = val_s
    mask1[flat] = 1.0
    # dst slots (last slot of each block)
    posn = np.arange(npos1)
    dv1 = np.zeros(npos1, np.int16)
    dv1[:K] = uindex[spec_by_pos]
    fdst = (slot_base1[posn // P] + np.asarray(L1)[posn // P] - 1) * P + posn % P
    idx1[fdst] = dv1
    dp1 = np.zeros(npos1, np.float32)
    dp1[:K] = 1.0
    l1_eidx = _wrap16(idx1)
    l1_f = np.concatenate(
        [np.ascontiguousarray(mask1.reshape(S1, P).T),
         np.ascontiguousarray(dp1.reshape(nblk1, P).T)], axis=1)
    groups1 = _groups_of(L1)

    # ---- layer 2 structure (sparse-special grid) ----
    npos = ((NPC + P - 1) // P) * P
    nblk2 = npos // P
    ROW_B2 = K + 1
    REPL_LO = K + 2
    REPL_HI = REPL_LO + NREPL
    assert REPL_HI <= VTAB
    percore = []
    for c in range(NCORES):
        sel = (d2 >= c * NPC) & (d2 < (c + 1) * NPC)
        dl = d2[sel] - c * NPC
        sl = s2[sel]
        deg = np.bincount(dl, minlength=NPC)
        spr_all = rowmap[sl]
        m = spr_all > 0
        spd = dl[m]
        spr = spr_all[m]
        deg_sp = np.bincount(spd, minlength=NPC)
        ndef = deg - deg_sp
        spadst = rowmap[c * NPC:(c + 1) * NPC] > 0
        front = spadst | (deg == 0)
        assert int(front.sum()) <= P
        key = front.astype(np.int64) * (1 << 20) + deg_sp
        order = np.argsort(-key, kind="stable")
        ncomp = int((key > 0).sum())
        percore.append(dict(deg=deg, deg_sp=deg_sp, ndef=ndef, spd=spd,
                            spr=spr, order=order, ncomp=ncomp))
    ncompb = max(1, max((pc["ncomp"] + P - 1) // P for pc in percore))
    assert ncompb * P <= NPC

    def blkmax(pc, b):
        return int(pc["deg_sp"][pc["order"][b * P:(b + 1) * P]].max())

    L0 = max(blkmax(pc, 0) for pc in percore) + 2   # +default +dst slot
    Lb = [max(max(blkmax(pc, b) for pc in percore), 1)
          for b in range(1, ncompb)]
    S_g = L0 + int(sum(Lb))
    slot_base = np.zeros(ncompb, np.int64)                 # per-block slot base
    if ncompb > 1:
        slot_base[1:] = L0 + np.concatenate([[0], np.cumsum(Lb)[:-1]])
    cap = np.asarray([L0 - 2] + Lb)                        # special capacity
    groups2b = _groups_of(Lb, b0=1)

    cores = []
    nposc = ncompb * P
    for c, pc in enumerate(percore):
        deg, deg_sp, ndef = pc["deg"], pc["deg_sp"], pc["ndef"]
        spd, spr, order = pc["spd"], pc["spr"], pc["order"]
        rng = np.random.default_rng(1000 + c)
        idxflat = rng.integers(REPL_LO, REPL_HI, S_g * P).astype(np.int16)
        maskflat = np.zeros(S_g * P, np.float32)
        wts0 = np.zeros(L0 * P, np.float32)
        rank = np.empty(NPC, np.int64)
        rank[order] = np.arange(NPC)
        # special-edge slots
        pos = rank[spd]
        pe = np.argsort(pos, kind="stable")
        pos_s = pos[pe]
        val_s = spr[pe].astype(np.int16)
        assert pos_s.size == 0 or pos_s.max() < nposc
        start_of_pos = np.searchsorted(pos_s, np.arange(nposc))
        kk = np.arange(len(pos_s)) - start_of_pos[pos_s]
        assert np.all(kk < cap[pos_s // P])
        flat = (slot_base[pos_s // P] + kk) * P + (pos_s % P)
        idxflat[flat] = val_s
        maskflat[flat] = 1.0
        # block 0: default + dst slots
        p0 = np.arange(P)
        nodes0 = order[p0]
        nd0 = ndef[nodes0].astype(np.float32)
        is00 = deg[nodes0] == 0
        fd = (L0 - 2) * P + p0
        ft = (L0 - 1) * P + p0
        maskflat[fd] = ((nd0 > 0) | is00).astype(np.float32)
        wts0[0:(L0 - 2) * P] = maskflat[0:(L0 - 2) * P]
        wts0[fd] = np.where(is00, 1.0, nd0)
        idxflat[fd] = np.where(is00, np.int16(ROW_B2), idxflat[fd])
        rm0 = rowmap[c * NPC + nodes0]
        h0 = rm0 > 0
        idxflat[ft[h0]] = rm0[h0]
        # blocks >= 1 scalar fields
        posn = np.arange(P, nposc)
        nodesb = order[posn]
        assert np.all(deg[nodesb] > 0)
        m0b = (ndef[nodesb] > 0).astype(np.float32)
        w0b = ndef[nodesb].astype(np.float32)
        m0b_t = np.ascontiguousarray(m0b.reshape(ncompb - 1, P).T) \
            if ncompb > 1 else np.zeros((P, 0), np.float32)
        w0b_t = np.ascontiguousarray(w0b.reshape(ncompb - 1, P).T) \
            if ncompb > 1 else np.zeros((P, 0), np.float32)
        l2f = np.concatenate(
            [np.ascontiguousarray(maskflat.reshape(S_g, P).T),
             np.ascontiguousarray(wts0.reshape(L0, P).T),
             m0b_t, w0b_t], axis=1)
        cores.append({"eidx2": _wrap16(idxflat), "l2f": l2f, "order": order})

    meta = {
        "K": K, "nblk1": nblk1, "nU": nU, "nUt": nUt, "xranges": xranges,
        "L1": L1, "groups1": groups1, "S1": S1,
        "L0": L0, "Lb": Lb, "S_g": S_g, "groups2b": groups2b,
        "ncompb": ncompb, "nblk2": nblk2, "npos": npos,
        "ROW_B2": ROW_B2, "REPL_LO": REPL_LO,
    }
    l1 = {"uidx16": uidx16, "l1_eidx": l1_eidx, "l1_f": l1_f}
    return meta, l1, cores


def _extract_lastslot(nc, gw, Gap, B, L, col, tag):
    """[P, B] tile holding Gap[:, b*L + L-1, col] per block b."""
    t = gw.tile([P, B], F32, tag=tag)
    nc.scalar.activation(
        t[:],
        Gap[:, :, col:col + 1]
        .rearrange("p (b l) o -> p b (l o)", l=L)[:, :, L - 1:L]
        .rearrange("p b o -> p (b o)"),
        AF.Identity)
    return t


def _emit_group(nc, gw, Gap, mask_ap, adst_ap, B, L, wts_ap=None,
                degpos_ap=None, out_ap=None):
    """Segment softmax + weighted sum for B blocks of equal padded degree L.

    Gap: AP view [128, B*L, 128] of the gathered rows (slot-flat).
    Returns msg tile [128, B, 64]."""
    BL = B * L
    asrc = Gap[:, :, 64:65].rearrange("p s o -> p (s o)")        # [128, BL]
    s_t = gw.tile([P, B, L], F32, tag="s_t")
    nc.vector.tensor_tensor(s_t[:], asrc, adst_ap.to_broadcast((P, B, L)),
                            op=OP.add)
    u_t = gw.tile([P, B, L], F32, tag="u_t")
    nc.vector.scalar_tensor_tensor(u_t[:], s_t[:], NEG_SLOPE, s_t[:],
                                   op0=OP.mult, op1=OP.max)
    e2_t = gw.tile([P, B, L], F32, tag="e2_t")
    nc.vector.scalar_tensor_tensor(e2_t[:], u_t[:], BIG, mask_ap,
                                   op0=OP.add, op1=OP.mult)
    mneg = gw.tile([P, B], F32, tag="mneg")
    nc.vector.tensor_reduce(mneg[:], e2_t[:], axis=mybir.AxisListType.X,
                            op=OP.max, negate=True)
    d_t = gw.tile([P, B, L], F32, tag="d_t")
    nc.vector.tensor_tensor(d_t[:], e2_t[:], mneg[:].to_broadcast((P, B, L)),
                            op=OP.add)
    ex_t = gw.tile([P, B, L], F32, tag="ex_t")
    nc.scalar.activation(ex_t[:], d_t[:], AF.Exp)
    if wts_ap is not None:
        exw_t = gw.tile([P, B, L], F32, tag="exw_t")
        nc.vector.tensor_tensor(exw_t[:], ex_t[:], wts_ap, op=OP.mult)
    else:
        exw_t = ex_t
    ssum = gw.tile([P, B], F32, tag="ssum")
    nc.vector.tensor_reduce(ssum[:], exw_t[:], axis=mybir.AxisListType.X,
                            op=OP.add)
    rs = gw.tile([P, B], F32, tag="rs")
    nc.vector.reciprocal(rs[:], ssum[:])
    if degpos_ap is not None:
        rsd = gw.tile([P, B], F32, tag="rsd")
        nc.vector.tensor_tensor(rsd[:], rs[:], degpos_ap, op=OP.mult)
    else:
        rsd = rs
    alpha = gw.tile([P, B, L], F32, tag="alpha")
    nc.vector.tensor_tensor(alpha[:], exw_t[:], rsd[:].to_broadcast((P, B, L)),
                            op=OP.mult)
    wr = gw.tile([P, BL, D], F32, tag="wr")
    nc.vector.tensor_tensor(wr[:], Gap[:, :, 0:D],
                            alpha[:].rearrange("p b l -> p (b l)")
                            .to_broadcast((P, BL, D)), op=OP.mult)
    if out_ap is None:
        msg = gw.tile([P, B, D], F32, tag="msg")
        out_ap = msg[:]
    else:
        msg = None
    nc.vector.tensor_reduce(out_ap,
                            wr[:].rearrange("p (b l) f -> p b f l", b=B),
                            axis=mybir.AxisListType.X, op=OP.add)
    return msg


def _emit_sp_group(nc, gw, Gap, mask_ap, m0b_ap, w0b_ap, e0c, cdrep, defrowv,
                   B, L, out_ap=None):
    """Blocks with only special-edge slots: default-edge mass via scalars.

    Gap [128, B*L, 128]; adst = c_d (non-special dsts); e0c [P,1] = default
    score lrelu(c_s+c_d)+BIG; m0b/w0b [P,B] = (ndef>0) and ndef.
    Returns msg tile [128, B, 64] (includes the default-row contribution)."""
    BL = B * L
    asrc = Gap[:, :, 64:65].rearrange("p s o -> p (s o)")
    s_t = gw.tile([P, B, L], F32, tag="sp_s")
    nc.vector.tensor_tensor(s_t[:], asrc, cdrep.to_broadcast((P, B, L)),
                            op=OP.add)
    u_t = gw.tile([P, B, L], F32, tag="sp_u")
    nc.vector.scalar_tensor_tensor(u_t[:], s_t[:], NEG_SLOPE, s_t[:],
                                   op0=OP.mult, op1=OP.max)
    e2_t = gw.tile([P, B, L], F32, tag="sp_e2")
    nc.vector.scalar_tensor_tensor(e2_t[:], u_t[:], BIG, mask_ap,
                                   op0=OP.add, op1=OP.mult)
    e0e = gw.tile([P, B], F32, tag="sp_e0")
    nc.vector.tensor_tensor(e0e[:], m0b_ap, e0c.to_broadcast((P, B)),
                            op=OP.mult)
    m_t = gw.tile([P, B], F32, tag="sp_m")
    if L > 1:
        msp = gw.tile([P, B], F32, tag="sp_msp")
        nc.vector.tensor_reduce(msp[:], e2_t[:], axis=mybir.AxisListType.X,
                                op=OP.max)
        nc.vector.tensor_tensor(m_t[:], msp[:], e0e[:], op=OP.max)
    else:
        nc.vector.tensor_tensor(
            m_t[:], e2_t[:].rearrange("p b l -> p (b l)"), e0e[:], op=OP.max)
    d_t = gw.tile([P, B, L], F32, tag="sp_d")
    nc.vector.tensor_tensor(d_t[:], e2_t[:], m_t[:].to_broadcast((P, B, L)),
                            op=OP.subtract)
    ex_t = gw.tile([P, B, L], F32, tag="sp_ex")
    nc.scalar.activation(ex_t[:], d_t[:], AF.Exp)
    d0 = gw.tile([P, B], F32, tag="sp_d0")
    nc.vector.tensor_tensor(d0[:], e0e[:], m_t[:], op=OP.subtract)
    ex0 = gw.tile([P, B], F32, tag="sp_ex0")
    nc.scalar.activation(ex0[:], d0[:], AF.Exp)
    exw0 = gw.tile([P, B], F32, tag="sp_exw0")
    nc.vector.tensor_tensor(exw0[:], ex0[:], w0b_ap, op=OP.mult)
    ssum = gw.tile([P, B], F32, tag="sp_ssum")
    if L > 1:
        ssp = gw.tile([P, B], F32, tag="sp_ssp")
        nc.vector.tensor_reduce(ssp[:], ex_t[:], axis=mybir.AxisListType.X,
                                op=OP.add)
        nc.vector.tensor_tensor(ssum[:], ssp[:], exw0[:], op=OP.add)
    else:
        nc.vector.tensor_tensor(
            ssum[:], ex_t[:].rearrange("p b l -> p (b l)"), exw0[:], op=OP.add)
    rs = gw.tile([P, B], F32, tag="sp_rs")
    nc.vector.reciprocal(rs[:], ssum[:])
    alpha = gw.tile([P, B, L], F32, tag="sp_al")
    nc.vector.tensor_tensor(alpha[:], ex_t[:], rs[:].to_broadcast((P, B, L)),
                            op=OP.mult)
    alpha0 = gw.tile([P, B], F32, tag="sp_al0")
    nc.vector.tensor_tensor(alpha0[:], exw0[:], rs[:], op=OP.mult)
    wr = gw.tile([P, BL, D], F32, tag="sp_wr")
    nc.vector.tensor_tensor(wr[:], Gap[:, :, 0:D],
                            alpha[:].rearrange("p b l -> p (b l)")
                            .to_broadcast((P, BL, D)), op=OP.mult)
    if L > 1:
        msgs = gw.tile([P, B, D], F32, tag="sp_msgs")
        nc.vector.tensor_reduce(
            msgs[:], wr[:].rearrange("p (b l) f -> p b f l", b=B),
            axis=mybir.AxisListType.X, op=OP.add)
    else:
        msgs = wr
    t1 = gw.tile([P, B, D], F32, tag="sp_t1")
    nc.vector.tensor_tensor(t1[:], alpha0[:].to_broadcast((P, B, D)),
                            defrowv.to_broadcast((P, B, D)), op=OP.mult)
    if out_ap is None:
        msg = gw.tile([P, B, D], F32, tag="sp_msg")
        out_ap = msg[:]
    else:
        msg = None
    nc.vector.tensor_tensor(out_ap, msgs[:], t1[:], op=OP.add)
    return msg


def build(meta, repeat=1, stages="ducge"):
    """Build the SPMD Bass program (common across cores).

    stages: subset of 'd' (default writes), 'u' (l1 endpoint table),
    'c' (l1 conv -> tab rows), 'g' (l2 gather), 'e' (l2 emit+write)."""
    K = meta["K"]
    nblk1, nUt = meta["nblk1"], meta["nUt"]
    S1, groups1, L1 = meta["S1"], meta["groups1"], meta["L1"]
    S_g, L0, groups2b = meta["S_g"], meta["L0"], meta["groups2b"]
    ncompb, nblk2 = meta["ncompb"], meta["nblk2"]
    ROW_B2, REPL_LO = meta["ROW_B2"], meta["REPL_LO"]
    NPRM = 4 * D + 5
    nf1 = S1 + nblk1
    nf2 = S_g + L0 + 2 * (ncompb - 1)

    nc = bacc.Bacc("TRN2", target_bir_lowering=False, debug=False,
                   num_devices=NCORES)
    dt = nc.dram_tensor
    xranges = meta["xranges"]
    x_in = dt("x_in", [N, D], F32, kind="ExternalInput").ap()
    params_in = dt("params_in", [D, NPRM], F32, kind="ExternalInput").ap()
    b2row_in = dt("b2row_in", [1, D], F32, kind="ExternalInput").ap()
    uidx_in = dt("uidx_in", [P, 8 * nUt], I16, kind="ExternalInput").ap()
    l1_eidx_in = dt("l1_eidx_in", [P, 8 * S1], I16, kind="ExternalInput").ap()
    l1_f_in = dt("l1_f_in", [P, nf1], F32, kind="ExternalInput").ap()
    eidx2_in = dt("eidx2_in", [P, 8 * S_g], I16, kind="ExternalInput").ap()
    l2f_in = dt("l2f_in", [P, nf2], F32, kind="ExternalInput").ap()
    out_t = dt("out", [meta["npos"], D], F32, kind="ExternalOutput").ap()

    h1tab = dt("h1tab", [nUt * P, P], F32).ap()
    tab = dt("tab", [VTAB, P], F32).ap()

    with tile.TileContext(nc) as tc, ExitStack() as ctx:
        const = ctx.enter_context(tc.tile_pool(name="const", bufs=1))
        psc_ctx = tc.tile_pool(name="psc", bufs=1, space="PSUM")
        psc = psc_ctx.__enter__()

        ident = const.tile([P, P], F32)
        make_identity(nc, ident[:])

        # ---- parameters (one blob: W1|W1T|W2|W2T|av1|av2|b1col) ----
        prm = const.tile([D, NPRM], F32)
        nc.sync.dma_start(prm[:], params_in[:])
        W1s = prm[:, 0:D]
        W1Ts = prm[:, D:2 * D]
        W2s = prm[:, 2 * D:3 * D]
        W2Ts = prm[:, 3 * D:4 * D]
        av1s = prm[:, 4 * D:4 * D + 2]
        av2s = prm[:, 4 * D + 2:4 * D + 4]
        b1cols = prm[:, 4 * D + 4:4 * D + 5]
        b2rows = const.tile([1, D], F32)
        nc.sync.dma_start(b2rows[:], b2row_in[:])

        wt1_p = psc.tile([D, 2], F32, space="PSUM")
        nc.tensor.matmul(wt1_p[:], W1Ts, av1s, start=True, stop=True)
        wt2_p = psc.tile([D, 2], F32, space="PSUM")
        nc.tensor.matmul(wt2_p[:], W2Ts, av2s, start=True, stop=True)
        wt2s = const.tile([D, 2], F32)
        nc.vector.tensor_copy(wt2s[:], wt2_p[:])

        W1aug = const.tile([D, D + 2], F32)
        nc.vector.tensor_copy(W1aug[:, 0:D], W1s)
        nc.vector.tensor_copy(W1aug[:, D:D + 2], wt1_p[:])

        # SPEC2 [65, 66] = [[W2 | wt2s wt2d]; [b1@W2+b2 | b1.wt2s b1.wt2d]]
        SPEC = const.tile([D + 1, D + 2], F32)
        nc.vector.tensor_copy(SPEC[0:D, 0:D], W2s)
        nc.vector.tensor_copy(SPEC[0:D, D:D + 2], wt2s[:])
        b1w2_p = psc.tile([1, D], F32, space="PSUM")
        nc.tensor.matmul(b1w2_p[:], b1cols, W2s, start=True, stop=True)
        nc.vector.tensor_tensor(SPEC[D:D + 1, 0:D], b1w2_p[:], b2rows[:],
                                op=OP.add)
        b1w_p = psc.tile([1, 2], F32, space="PSUM")
        nc.tensor.matmul(b1w_p[:], b1cols, wt2s[:], start=True, stop=True)
        nc.vector.tensor_copy(SPEC[D:D + 1, D:D + 2], b1w_p[:])

        # block-diagonal pair matrices (two 64-wide tiles per PE pass)
        W1aug2 = const.tile([2 * D, 2 * (D + 2)], F32)
        nc.vector.memset(W1aug2[:], 0.0)
        nc.vector.tensor_copy(W1aug2[0:D, 0:D + 2], W1aug[:])
        nc.vector.tensor_copy(W1aug2[D:2 * D, D + 2:2 * (D + 2)], W1aug[:])
        W2D = const.tile([2 * D, 2 * (D + 2)], F32)
        nc.vector.memset(W2D[:], 0.0)
        nc.vector.tensor_copy(W2D[0:D, 0:D + 2], SPEC[0:D, :])
        nc.vector.tensor_copy(W2D[D:2 * D, D + 2:2 * (D + 2)], SPEC[0:D, :])

        # ---- constant table rows + replicated default row ----
        row0_s = const.tile([1, P], F32)
        nc.vector.memset(row0_s[:], 0.0)
        nc.vector.tensor_copy(row0_s[:, 0:D + 2], SPEC[D:D + 1, :])
        b2r_s = const.tile([1, P], F32)
        nc.vector.memset(b2r_s[:], 0.0)
        nc.vector.tensor_copy(b2r_s[:, 0:D], b2rows[:])
        ones_s = const.tile([1, P], F32)
        nc.vector.memset(ones_s[:], 1.0)
        repl_p = psc.tile([P, P], F32, space="PSUM")
        nc.tensor.matmul(repl_p[:], ones_s[:], row0_s[:], start=True, stop=True)
        repl_s = const.tile([P, P], F32)
        nc.vector.tensor_copy(repl_s[:], repl_p[:])
        defrowv = repl_s[:, 0:D].rearrange("p (k f) -> p k f", k=1)
        csrep = repl_s[:, D:D + 1]
        cdrep = repl_s[:, D + 1:D + 2]
        replv = repl_s[:].rearrange("p (k f) -> p k f", k=1)
        KB = 16                      # blocks per default-write chunk
        defbig = const.tile([P, KB * D], F32)
        nc.vector.tensor_copy(
            defbig[:].rearrange("p (k f) -> p k f", k=KB),
            defrowv.to_broadcast((P, KB, D)))

        nc.sync.dma_start(tab[0:1, :], row0_s[:])
        nc.sync.dma_start(tab[ROW_B2:ROW_B2 + 1, :], b2r_s[:])
        nc.sync.dma_start(
            tab[REPL_LO:REPL_LO + NREPL, :].rearrange("(k p) f -> p k f", p=P),
            replv.to_broadcast((P, NREPL // P, P)))

        # default-score constant e0c = lrelu(c_s + c_d) + BIG
        s0c = const.tile([P, 1], F32)
        nc.vector.tensor_tensor(s0c[:], csrep, cdrep, op=OP.add)
        u0c = const.tile([P, 1], F32)
        nc.vector.scalar_tensor_tensor(u0c[:], s0c[:], NEG_SLOPE, s0c[:],
                                       op0=OP.mult, op1=OP.max)
        e0c = const.tile([P, 1], F32)
        nc.vector.tensor_scalar_add(e0c[:], u0c[:], BIG)

        psc_ctx.__exit__(None, None, None)

        # ---- index tensors ----
        uidx_s = const.tile([P, 8 * nUt], I16)
        nc.sync.dma_start(uidx_s[:], uidx_in[:])
        l1_eidx_s = const.tile([P, 8 * S1], I16)
        nc.sync.dma_start(l1_eidx_s[:], l1_eidx_in[:])
        l1_f_s = const.tile([P, nf1], F32)
        nc.sync.dma_start(l1_f_s[:], l1_f_in[:])
        l1_mask_s = l1_f_s[:, 0:S1]
        l1_degpos_s = l1_f_s[:, S1:S1 + nblk1]
        eidx2_s = const.tile([P, 8 * S_g], I16)
        nc.sync.dma_start(eidx2_s[:], eidx2_in[:])
        l2f_s = const.tile([P, nf2], F32)
        nc.sync.dma_start(l2f_s[:], l2f_in[:])
        mask2_s = l2f_s[:, 0:S_g]
        wts0_s = l2f_s[:, S_g:S_g + L0]
        m0b_s = l2f_s[:, S_g + L0:S_g + L0 + (ncompb - 1)]
        w0b_s = l2f_s[:, S_g + L0 + (ncompb - 1):nf2]

        # persistent staging tiles (values rewritten every rep)
        h_all = const.tile([P, nUt, D + 2], F32)
        npr1 = nblk1 // 2
        mTs_c = const.tile([2 * D, max(npr1, 1), P], F32)
        mTs_s = const.tile([D, P], F32)
        repl2 = const.tile([P, 2 * (D + 2)], F32)
        nc.vector.tensor_copy(repl2[:, 0:D + 2], repl_s[:, 0:D + 2])
        nc.vector.tensor_copy(repl2[:, D + 2:2 * (D + 2)],
                              repl_s[:, 0:D + 2])
        row_all = const.tile([P, nblk1, D + 2], F32)
        msg1_all = const.tile([P, nblk1, D], F32)
        msgall = const.tile([P, ncompb, D], F32)

        ndefblk = nblk2 - ncompb
        outcmp = out_t[0:ncompb * P, :].rearrange("(p b) f -> p b f", b=ncompb)

        for _rep in range(repeat):
            # ---- default-region output: chunked contiguous writes ----
            if "d" in stages:
                b = 0
                while b < ndefblk:
                    nb = min(KB, ndefblk - b)
                    r0 = (ncompb + b) * P
                    nc.sync.dma_start(
                        out_t[r0:r0 + nb * P, :].rearrange(
                            "(p k) f -> p (k f)", k=nb),
                        defbig[:, 0:nb * D])
                    b += nb

            # ---- layer 1: h1 table for the U endpoint nodes ----
            if "u" not in stages:
                continue
            with tc.tile_pool(name="l1u", bufs=2) as l1u, \
                 tc.tile_pool(name="l1up", bufs=4, space="PSUM") as l1up:
                xall = l1u.tile([P, nUt, D], F32, tag="xall")
                toff = 0
                for lo, nt in xranges:
                    hi = min(lo + (1 << 15), N)
                    nc.gpsimd.dma_gather(
                        xall[:, toff:toff + nt, :], x_in[lo:hi, :],
                        uidx_s[:, 8 * toff:8 * (toff + nt)],
                        nt * P, nt * P, D, single_packet=False)
                    toff += nt
                for t2 in range(nUt // 2):
                    xT_p = l1up.tile([P, P], F32, space="PSUM", tag="xT")
                    nc.tensor.transpose(
                        xT_p[:],
                        xall[:, 2 * t2:2 * t2 + 2, :]
                        .rearrange("p k f -> p (k f)"), ident[:])
                    xT_s = l1u.tile([P, P], F32, tag="xTs")
                    nc.vector.tensor_copy(xT_s[:], xT_p[:])
                    h_p = l1up.tile([P, 2 * (D + 2)], F32, space="PSUM",
                                    tag="h_p")
                    nc.tensor.matmul(h_p[:], xT_s[:], W1aug2[:], start=True,
                                     stop=True)
                    nc.scalar.copy(
                        h_all[:, 2 * t2:2 * t2 + 2, :]
                        .rearrange("p k f -> p (k f)"), h_p[:])
                if nUt % 2:
                    t = nUt - 1
                    xT_p1 = l1up.tile([D, P], F32, space="PSUM", tag="xT1")
                    nc.tensor.transpose(xT_p1[:], xall[:, t, :], ident[:])
                    xT_s1 = l1u.tile([D, P], F32, tag="xTs1")
                    nc.vector.tensor_copy(xT_s1[:], xT_p1[:])
                    h_p1 = l1up.tile([P, D + 2], F32, space="PSUM", tag="h_p1")
                    nc.tensor.matmul(h_p1[:], xT_s1[:], W1aug[:], start=True,
                                     stop=True)
                    nc.scalar.copy(h_all[:, t, :], h_p1[:])
                nc.sync.dma_start(
                    h1tab[:, 0:D + 2].rearrange("(k p) f -> p k f", p=P),
                    h_all[:])

            # ---- layer 1 conv -> write special table rows 1..K ----
            if "c" not in stages:
                continue
            with tc.tile_pool(name="l1w", bufs=2) as l1w, \
                 tc.tile_pool(name="l1p", bufs=4, space="PSUM") as l1p:
                G1 = l1w.tile([P, S1, P], F32, tag="G1")
                nc.gpsimd.dma_gather(G1[:], h1tab[:, :], l1_eidx_s[:],
                                     S1 * P, S1 * P, P, single_packet=False)
                for g in groups1:
                    B, L, off = g["B"], g["L"], g["slot_off"]
                    Gap = G1[:, off:off + B * L, :]
                    adst1 = _extract_lastslot(nc, l1w, Gap, B, L, D + 1,
                                              "adst1")
                    _emit_group(
                        nc, l1w, Gap, l1_mask_s[:, off:off + B * L],
                        adst1[:], B, L,
                        degpos_ap=l1_degpos_s[:, g["b0"]:g["b0"] + B],
                        out_ap=msg1_all[:, g["b0"]:g["b0"] + B, :])
                for pr in range(nblk1 // 2):
                    mT_p = l1p.tile([P, P], F32, space="PSUM", tag="mT")
                    nc.tensor.transpose(
                        mT_p[:],
                        msg1_all[:, 2 * pr:2 * pr + 2, :]
                        .rearrange("p k f -> p (k f)"), ident[:])
                    nc.vector.tensor_copy(mTs_c[:, pr, :], mT_p[:])
                    row_p = l1p.tile([P, 2 * (D + 2)], F32, space="PSUM",
                                     tag="rowp")
                    nc.tensor.matmul(row_p[:], mTs_c[:, pr, :], W2D[:],
                                     start=True, stop=True)
                    nc.vector.tensor_tensor(
                        row_all[:, 2 * pr:2 * pr + 2, :]
                        .rearrange("p k f -> p (k f)"), row_p[:], repl2[:],
                        op=OP.add)
                if nblk1 % 2:
                    b = nblk1 - 1
                    mT_p1 = l1p.tile([D, P], F32, space="PSUM", tag="mT1")
                    nc.tensor.transpose(mT_p1[:], msg1_all[:, b, :], ident[:])
                    nc.vector.tensor_copy(mTs_s[:], mT_p1[:])
                    row_p1 = l1p.tile([P, D + 2], F32, space="PSUM",
                                      tag="rowp1")
                    nc.tensor.matmul(row_p1[:], mTs_s[:], SPEC[0:D, :],
                                     start=True, stop=True)
                    nc.vector.tensor_tensor(row_all[:, b, :], row_p1[:],
                                            repl2[:, 0:D + 2], op=OP.add)
                nfull = K // P
                if nfull:
                    nc.sync.dma_start(
                        tab[1:1 + nfull * P, 0:D + 2].rearrange(
                            "(k p) f -> p k f", p=P),
                        row_all[:, 0:nfull, :])
                rem = K - nfull * P
                if rem:
                    nc.sync.dma_start(tab[1 + nfull * P:1 + K, 0:D + 2],
                                      row_all[0:rem, nfull, :])

            # ---- layer 2 ----
            if "g" not in stages:
                continue
            with tc.tile_pool(name="gw", bufs=2) as gw:
                G = gw.tile([P, S_g, P], F32, tag="G")
                nc.gpsimd.dma_gather(G[:], tab[:, :], eidx2_s[:],
                                     S_g * P, S_g * P, P, single_packet=False)
                if "e" not in stages:
                    dum = gw.tile([P, P], F32, tag="dum")
                    nc.vector.tensor_copy(dum[:], G[:, 0, :])
                    continue
                # block 0: full grid
                Gap0 = G[:, 0:L0, :]
                adst0 = _extract_lastslot(nc, gw, Gap0, 1, L0, D + 1, "adst0")
                _emit_group(nc, gw, Gap0, mask2_s[:, 0:L0], adst0[:],
                            1, L0, wts_ap=wts0_s[:],
                            out_ap=msgall[:, 0:1, :])
                # blocks >= 1: special-only grids
                for g in groups2b:
                    B, L, off = g["B"], g["L"], g["slot_off"]
                    Gap = G[:, L0 + off:L0 + off + B * L, :]
                    _emit_sp_group(
                        nc, gw, Gap, mask2_s[:, L0 + off:L0 + off + B * L],
                        m0b_s[:, g["b0"] - 1:g["b0"] - 1 + B],
                        w0b_s[:, g["b0"] - 1:g["b0"] - 1 + B],
                        e0c[:], cdrep, defrowv, B, L,
                        out_ap=msgall[:, g["b0"]:g["b0"] + B, :])
                nc.sync.dma_start(outcmp, msgall[:])

    nc.compile()
    return nc


def make_in_maps(inputs, meta, l1, cores):
    x = np.ascontiguousarray(np.asarray(inputs["x"], dtype=np.float32))
    W1 = np.asarray(inputs["W1"], dtype=np.float32)
    W2 = np.asarray(inputs["W2"], dtype=np.float32)
    params = np.concatenate(
        [W1, np.ascontiguousarray(W1.T), W2, np.ascontiguousarray(W2.T),
         np.stack([np.asarray(inputs["a_src1"]),
                   np.asarray(inputs["a_dst1"])], axis=1),
         np.stack([np.asarray(inputs["a_src2"]),
                   np.asarray(inputs["a_dst2"])], axis=1),
         np.asarray(inputs["b1"]).reshape(D, 1)],
        axis=1).astype(np.float32)
    base = {
        "x_in": x,
        "params_in": np.ascontiguousarray(params),
        "b2row_in": np.asarray(inputs["b2"], dtype=np.float32).reshape(1, D),
        "uidx_in": l1["uidx16"],
        "l1_eidx_in": l1["l1_eidx"],
        "l1_f_in": l1["l1_f"],
    }
    in_maps = []
    for c in range(NCORES):
        m = dict(base)
        m["eidx2_in"] = cores[c]["eidx2"]
        m["l2f_in"] = cores[c]["l2f"]
        in_maps.append(m)
    return in_maps


def unshard_core(oc, order, ncompb):
    got = np.empty((NPC, D), np.float32)
    nposc = ncompb * P
    pos = np.arange(nposc)
    got[order[:nposc]] = oc[(pos % P) * ncompb + pos // P]
    got[order[nposc:NPC]] = oc[nposc:NPC]
    return got


def unshard(results, cores, meta):
    out = np.empty((N, D), np.float32)
    for c in range(NCORES):
        out[c * NPC:(c + 1) * NPC] = unshard_core(
            results[c]["out"], cores[c]["order"], meta["ncompb"])
    return out


def kernel(**inputs):
    meta, l1, cores = prep(inputs)
    nc = build(meta, repeat=1)
    in_maps = make_in_maps(inputs, meta, l1, cores)
    res = run_bass_kernel_spmd(nc, in_maps, core_ids=list(range(NCORES)))
    return unshard(res.results, cores, meta)


# revision 33
# speedup vs baseline: 2.3625x; 2.3625x over previous
"""GAT 2-layer encoder on 8 Trainium2 NeuronCores.

Reference computation: layer 1 = GAT conv over edge_index[:, :500] (weights W1),
layer 2 = GAT conv over edge_index[:, 500:] (weights W2).

Strategy (sparse-special):
  - Layer-1 output x1 differs from b1 only on the K<=500 distinct dsts of the
    first 500 edges ("specials").  In layer 2, h2[src] = x1[src]@W2 is the
    constant default row for every non-special src, so only edges whose src is
    special (~8k of 1.6M) carry information.  For a dst with no special
    in-edge, softmax over equal scores gives alpha = 1/deg for every in-edge,
    hence out = b1@W2 + b2 exactly (up to the 1e-16 eps), a CONSTANT row.
  - Device builds a (K+2)-row table in h2-space with b2 baked in:
    row r = [x1_r@W2 + b2 | asrc2_r | adst2_r], row 0 = default, row K+1 = b2
    (for deg-0 dsts), rows K+2.. = replicas of row 0 (spread gather load).
  - Sharding: dst-range partition of the 1.6M layer-2 edges across 8 cores (no
    collectives; layer 1 + table build replicated on every core, it is tiny).
  - Per core: dsts sorted so that special-adst / deg-0 dsts land in block 0
    (full slot grid: special slots + default slot + dst slot, all gathered),
    remaining computed blocks carry ONLY special-edge slots; their default
    in-edge mass and adst2 = c_d are handled with per-position scalars.  The
    ~90 all-default blocks are written with one broadcast DMA of the constant
    row.  Layer-2 table rows are pre-transformed by W2, so no matmul there.
"""

import sys

sys.path.insert(0, "/opt/trn_rl_repo")

from contextlib import ExitStack

import numpy as np

import concourse.bacc as bacc
import concourse.bass as bass
import concourse.mybir as mybir
import concourse.tile as tile
from concourse.bass_utils import run_bass_kernel_spmd
from concourse.masks import make_identity

F32 = mybir.dt.float32
I16 = mybir.dt.int16
I32 = mybir.dt.int32
AF = mybir.ActivationFunctionType
OP = mybir.AluOpType

N = 100000
D = 64
NCORES = 8
NPC = N // NCORES          # dst nodes per core
P = 128
NSPLIT = 500               # first 500 edges -> layer 1
NEG_SLOPE = 0.2
BIG = 200.0                # score shift so padded slots underflow exp to 0.0
VTAB = 1024                # gather table rows (specials + default replicas)
NREPL = 512                # default-row replicas written (one broadcast DMA)


def _wrap16(flat):
    """int16 stream [n] (n%16==0) -> dma_gather idx tile [128, n//16]."""
    w = flat.reshape(-1, 16).T
    return np.ascontiguousarray(np.tile(w, (8, 1)).astype(np.int16))


def _groups_of(Ls, b0=0):
    """Split the per-block padded-degree profile into equal-L runs."""
    groups = []
    off = 0
    b = 0
    while b < len(Ls):
        s = b
        while b < len(Ls) and Ls[b] == Ls[s]:
            b += 1
        groups.append({"b0": b0 + s, "B": b - s, "L": Ls[s], "slot_off": off})
        off += (b - s) * Ls[s]
    return groups


def prep(inputs):
    """Host-side index prep (pure index computation, no feature values)."""
    ei = np.asarray(inputs["edge_index"])
    src = ei[0].astype(np.int64)
    dst = ei[1].astype(np.int64)
    s1, d1 = src[:NSPLIT], dst[:NSPLIT]
    s2, d2 = src[NSPLIT:], dst[NSPLIT:]

    # ---- layer 1 structure ----
    specials, deg1 = np.unique(d1, return_counts=True)
    K = len(specials)
    order1 = np.argsort(-deg1, kind="stable")
    spec_by_pos = specials[order1]          # grid position q -> node, table row q+1
    rowmap = np.zeros(N, np.int16)
    rowmap[spec_by_pos] = np.arange(1, K + 1)
    nblk1 = (K + P - 1) // P
    npos1 = nblk1 * P

    U = np.unique(np.concatenate([s1, d1]))
    nU = len(U)
    # x-row gather in int16-addressable ranges of 32768 rows
    RSPAN = 1 << 15
    xranges = []          # (lo, ntiles)
    uidx16_parts = []
    uindex = np.zeros(N, np.int64)
    off = 0
    for lo in range(0, N, RSPAN):
        hi = min(lo + RSPAN, N)
        Ur = U[(U >= lo) & (U < hi)]
        if len(Ur) == 0:
            continue
        nt = (len(Ur) + P - 1) // P
        pad = np.full(nt * P, lo, np.int64)
        pad[:len(Ur)] = Ur
        uindex[Ur] = off * P + np.arange(len(Ur))
        uidx16_parts.append(_wrap16((pad - lo).astype(np.int16)))
        xranges.append((lo, nt))
        off += nt
    nUt = off
    uidx16 = np.concatenate(uidx16_parts, axis=1)

    # layer-1 slot grid: per block [special-edge slots | dst slot]
    rank1 = np.empty(K, np.int64)
    rank1[order1] = np.arange(K)
    d1pos = rank1[np.searchsorted(specials, d1)]
    deg1_sorted = np.zeros(npos1, np.int64)
    deg1_sorted[:K] = deg1[order1]
    L1sp = [max(int(deg1_sorted[b * P:(b + 1) * P].max()), 1)
            for b in range(nblk1)]
    L1 = [l + 1 for l in L1sp]
    S1 = int(sum(L1))
    slot_base1 = np.concatenate([[0], np.cumsum(L1)])[:-1]
    idx1 = np.zeros(S1 * P, np.int16)
    mask1 = np.zeros(S1 * P, np.float32)
    pe = np.argsort(d1pos, kind="stable")
    pos_s = d1pos[pe]
    val_s = uindex[s1[pe]].astype(np.int16)
    start_of_pos = np.searchsorted(pos_s, np.arange(npos1))
    kk = np.arange(len(pos_s)) - start_of_pos[pos_s]
    flat = (slot_base1[pos_s // P] + kk) * P + (pos_s % P)
    idx1[flat] = val_s
    mask1[flat] = 1.0
    # dst slots (last slot of each block)
    posn = np.arange(npos1)
    dv1 = np.zeros(npos1, np.int16)
    dv1[:K] = uindex[spec_by_pos]
    fdst = (slot_base1[posn // P] + np.asarray(L1)[posn // P] - 1) * P + posn % P
    idx1[fdst] = dv1
    dp1 = np.zeros(npos1, np.float32)
    dp1[:K] = 1.0
    l1_eidx = _wrap16(idx1)
    l1_f = np.concatenate(
        [np.ascontiguousarray(mask1.reshape(S1, P).T),
         np.ascontiguousarray(dp1.reshape(nblk1, P).T)], axis=1)
    groups1 = _groups_of(L1)

    # ---- layer 2 structure (sparse-special grid) ----
    npos = ((NPC + P - 1) // P) * P
    nblk2 = npos // P
    ROW_B2 = K + 1
    REPL_LO = K + 2
    REPL_HI = REPL_LO + NREPL
    assert REPL_HI <= VTAB
    percore = []
    for c in range(NCORES):
        sel = (d2 >= c * NPC) & (d2 < (c + 1) * NPC)
        dl = d2[sel] - c * NPC
        sl = s2[sel]
        deg = np.bincount(dl, minlength=NPC)
        spr_all = rowmap[sl]
        m = spr_all > 0
        spd = dl[m]
        spr = spr_all[m]
        deg_sp = np.bincount(spd, minlength=NPC)
        ndef = deg - deg_sp
        spadst = rowmap[c * NPC:(c + 1) * NPC] > 0
        front = spadst | (deg == 0)
        assert int(front.sum()) <= P
        key = front.astype(np.int64) * (1 << 20) + deg_sp
        order = np.argsort(-key, kind="stable")
        ncomp = int((key > 0).sum())
        percore.append(dict(deg=deg, deg_sp=deg_sp, ndef=ndef, spd=spd,
                            spr=spr, order=order, ncomp=ncomp))
    ncompb = max(1, max((pc["ncomp"] + P - 1) // P for pc in percore))
    assert ncompb * P <= NPC

    def blkmax(pc, b):
        return int(pc["deg_sp"][pc["order"][b * P:(b + 1) * P]].max())

    L0 = max(blkmax(pc, 0) for pc in percore) + 2   # +default +dst slot
    Lb = [max(max(blkmax(pc, b) for pc in percore), 1)
          for b in range(1, ncompb)]
    S_g = L0 + int(sum(Lb))
    slot_base = np.zeros(ncompb, np.int64)                 # per-block slot base
    if ncompb > 1:
        slot_base[1:] = L0 + np.concatenate([[0], np.cumsum(Lb)[:-1]])
    cap = np.asarray([L0 - 2] + Lb)                        # special capacity
    groups2b = _groups_of(Lb, b0=1)

    cores = []
    nposc = ncompb * P
    for c, pc in enumerate(percore):
        deg, deg_sp, ndef = pc["deg"], pc["deg_sp"], pc["ndef"]
        spd, spr, order = pc["spd"], pc["spr"], pc["order"]
        rng = np.random.default_rng(1000 + c)
        idxflat = rng.integers(REPL_LO, REPL_HI, S_g * P).astype(np.int16)
        maskflat = np.zeros(S_g * P, np.float32)
        wts0 = np.zeros(L0 * P, np.float32)
        rank = np.empty(NPC, np.int64)
        rank[order] = np.arange(NPC)
        # special-edge slots
        pos = rank[spd]
        pe = np.argsort(pos, kind="stable")
        pos_s = pos[pe]
        val_s = spr[pe].astype(np.int16)
        assert pos_s.size == 0 or pos_s.max() < nposc
        start_of_pos = np.searchsorted(pos_s, np.arange(nposc))
        kk = np.arange(len(pos_s)) - start_of_pos[pos_s]
        assert np.all(kk < cap[pos_s // P])
        flat = (slot_base[pos_s // P] + kk) * P + (pos_s % P)
        idxflat[flat] = val_s
        maskflat[flat] = 1.0
        # block 0: default + dst slots
        p0 = np.arange(P)
        nodes0 = order[p0]
        nd0 = ndef[nodes0].astype(np.float32)
        is00 = deg[nodes0] == 0
        fd = (L0 - 2) * P + p0
        ft = (L0 - 1) * P + p0
        maskflat[fd] = ((nd0 > 0) | is00).astype(np.float32)
        wts0[0:(L0 - 2) * P] = maskflat[0:(L0 - 2) * P]
        wts0[fd] = np.where(is00, 1.0, nd0)
        idxflat[fd] = np.where(is00, np.int16(ROW_B2), idxflat[fd])
        rm0 = rowmap[c * NPC + nodes0]
        h0 = rm0 > 0
        idxflat[ft[h0]] = rm0[h0]
        # blocks >= 1 scalar fields
        posn = np.arange(P, nposc)
        nodesb = order[posn]
        assert np.all(deg[nodesb] > 0)
        m0b = (ndef[nodesb] > 0).astype(np.float32)
        w0b = ndef[nodesb].astype(np.float32)
        m0b_t = np.ascontiguousarray(m0b.reshape(ncompb - 1, P).T) \
            if ncompb > 1 else np.zeros((P, 0), np.float32)
        w0b_t = np.ascontiguousarray(w0b.reshape(ncompb - 1, P).T) \
            if ncompb > 1 else np.zeros((P, 0), np.float32)
        l2f = np.concatenate(
            [np.ascontiguousarray(maskflat.reshape(S_g, P).T),
             np.ascontiguousarray(wts0.reshape(L0, P).T),
             m0b_t, w0b_t], axis=1)
        cores.append({"eidx2": _wrap16(idxflat), "l2f": l2f, "order": order})

    meta = {
        "K": K, "nblk1": nblk1, "nU": nU, "nUt": nUt, "xranges": xranges,
        "L1": L1, "groups1": groups1, "S1": S1,
        "L0": L0, "Lb": Lb, "S_g": S_g, "groups2b": groups2b,
        "ncompb": ncompb, "nblk2": nblk2, "npos": npos,
        "ROW_B2": ROW_B2, "REPL_LO": REPL_LO,
    }
    l1 = {"uidx16": uidx16, "l1_eidx": l1_eidx, "l1_f": l1_f}
    return meta, l1, cores


def _extract_lastslot(nc, gw, Gap, B, L, col, tag):
    """[P, B] tile holding Gap[:, b*L + L-1, col] per block b."""
    t = gw.tile([P, B], F32, tag=tag)
    nc.scalar.activation(
        t[:],
        Gap[:, :, col:col + 1]
        .rearrange("p (b l) o -> p b (l o)", l=L)[:, :, L - 1:L]
        .rearrange("p b o -> p (b o)"),
        AF.Identity)
    return t


def _emit_group(nc, gw, Gap, mask_ap, adst_ap, B, L, wts_ap=None,
                degpos_ap=None, out_ap=None):
    """Segment softmax + weighted sum for B blocks of equal padded degree L.

    Gap: AP view [128, B*L, 128] of the gathered rows (slot-flat).
    Returns msg tile [128, B, 64]."""
    BL = B * L
    asrc = Gap[:, :, 64:65].rearrange("p s o -> p (s o)")        # [128, BL]
    s_t = gw.tile([P, B, L], F32, tag="s_t")
    nc.vector.tensor_tensor(s_t[:], asrc, adst_ap.to_broadcast((P, B, L)),
                            op=OP.add)
    u_t = gw.tile([P, B, L], F32, tag="u_t")
    nc.vector.scalar_tensor_tensor(u_t[:], s_t[:], NEG_SLOPE, s_t[:],
                                   op0=OP.mult, op1=OP.max)
    e2_t = gw.tile([P, B, L], F32, tag="e2_t")
    nc.vector.scalar_tensor_tensor(e2_t[:], u_t[:], BIG, mask_ap,
                                   op0=OP.add, op1=OP.mult)
    mneg = gw.tile([P, B], F32, tag="mneg")
    nc.vector.tensor_reduce(mneg[:], e2_t[:], axis=mybir.AxisListType.X,
                            op=OP.max, negate=True)
    d_t = gw.tile([P, B, L], F32, tag="d_t")
    nc.vector.tensor_tensor(d_t[:], e2_t[:], mneg[:].to_broadcast((P, B, L)),
                            op=OP.add)
    ex_t = gw.tile([P, B, L], F32, tag="ex_t")
    nc.scalar.activation(ex_t[:], d_t[:], AF.Exp)
    if wts_ap is not None:
        exw_t = gw.tile([P, B, L], F32, tag="exw_t")
        nc.vector.tensor_tensor(exw_t[:], ex_t[:], wts_ap, op=OP.mult)
    else:
        exw_t = ex_t
    ssum = gw.tile([P, B], F32, tag="ssum")
    nc.vector.tensor_reduce(ssum[:], exw_t[:], axis=mybir.AxisListType.X,
                            op=OP.add)
    rs = gw.tile([P, B], F32, tag="rs")
    nc.vector.reciprocal(rs[:], ssum[:])
    if degpos_ap is not None:
        rsd = gw.tile([P, B], F32, tag="rsd")
        nc.vector.tensor_tensor(rsd[:], rs[:], degpos_ap, op=OP.mult)
    else:
        rsd = rs
    alpha = gw.tile([P, B, L], F32, tag="alpha")
    nc.vector.tensor_tensor(alpha[:], exw_t[:], rsd[:].to_broadcast((P, B, L)),
                            op=OP.mult)
    wr = gw.tile([P, BL, D], F32, tag="wr")
    nc.vector.tensor_tensor(wr[:], Gap[:, :, 0:D],
                            alpha[:].rearrange("p b l -> p (b l)")
                            .to_broadcast((P, BL, D)), op=OP.mult)
    if out_ap is None:
        msg = gw.tile([P, B, D], F32, tag="msg")
        out_ap = msg[:]
    else:
        msg = None
    nc.vector.tensor_reduce(out_ap,
                            wr[:].rearrange("p (b l) f -> p b f l", b=B),
                            axis=mybir.AxisListType.X, op=OP.add)
    return msg


def _emit_sp_group(nc, gw, Gap, mask_ap, m0b_ap, w0b_ap, e0c, cdrep, defrowv,
                   B, L, out_ap=None):
    """Blocks with only special-edge slots: default-edge mass via scalars.

    Gap [128, B*L, 128]; adst = c_d (non-special dsts); e0c [P,1] = default
    score lrelu(c_s+c_d)+BIG; m0b/w0b [P,B] = (ndef>0) and ndef.
    Returns msg tile [128, B, 64] (includes the default-row contribution)."""
    BL = B * L
    asrc = Gap[:, :, 64:65].rearrange("p s o -> p (s o)")
    s_t = gw.tile([P, B, L], F32, tag="sp_s")
    nc.vector.tensor_tensor(s_t[:], asrc, cdrep.to_broadcast((P, B, L)),
                            op=OP.add)
    u_t = gw.tile([P, B, L], F32, tag="sp_u")
    nc.vector.scalar_tensor_tensor(u_t[:], s_t[:], NEG_SLOPE, s_t[:],
                                   op0=OP.mult, op1=OP.max)
    e2_t = gw.tile([P, B, L], F32, tag="sp_e2")
    nc.vector.scalar_tensor_tensor(e2_t[:], u_t[:], BIG, mask_ap,
                                   op0=OP.add, op1=OP.mult)
    e0e = gw.tile([P, B], F32, tag="sp_e0")
    nc.vector.tensor_tensor(e0e[:], m0b_ap, e0c.to_broadcast((P, B)),
                            op=OP.mult)
    m_t = gw.tile([P, B], F32, tag="sp_m")
    if L > 1:
        msp = gw.tile([P, B], F32, tag="sp_msp")
        nc.vector.tensor_reduce(msp[:], e2_t[:], axis=mybir.AxisListType.X,
                                op=OP.max)
        nc.vector.tensor_tensor(m_t[:], msp[:], e0e[:], op=OP.max)
    else:
        nc.vector.tensor_tensor(
            m_t[:], e2_t[:].rearrange("p b l -> p (b l)"), e0e[:], op=OP.max)
    d_t = gw.tile([P, B, L], F32, tag="sp_d")
    nc.vector.tensor_tensor(d_t[:], e2_t[:], m_t[:].to_broadcast((P, B, L)),
                            op=OP.subtract)
    ex_t = gw.tile([P, B, L], F32, tag="sp_ex")
    nc.scalar.activation(ex_t[:], d_t[:], AF.Exp)
    d0 = gw.tile([P, B], F32, tag="sp_d0")
    nc.vector.tensor_tensor(d0[:], e0e[:], m_t[:], op=OP.subtract)
    ex0 = gw.tile([P, B], F32, tag="sp_ex0")
    nc.scalar.activation(ex0[:], d0[:], AF.Exp)
    exw0 = gw.tile([P, B], F32, tag="sp_exw0")
    nc.vector.tensor_tensor(exw0[:], ex0[:], w0b_ap, op=OP.mult)
    ssum = gw.tile([P, B], F32, tag="sp_ssum")
    if L > 1:
        ssp = gw.tile([P, B], F32, tag="sp_ssp")
        nc.vector.tensor_reduce(ssp[:], ex_t[:], axis=mybir.AxisListType.X,
                                op=OP.add)
        nc.vector.tensor_tensor(ssum[:], ssp[:], exw0[:], op=OP.add)
    else:
        nc.vector.tensor_tensor(
            ssum[:], ex_t[:].rearrange("p b l -> p (b l)"), exw0[:], op=OP.add)
    rs = gw.tile([P, B], F32, tag="sp_rs")
    nc.vector.reciprocal(rs[:], ssum[:])
    alpha = gw.tile([P, B, L], F32, tag="sp_al")
    nc.vector.tensor_tensor(alpha[:], ex_t[:], rs[:].to_broadcast((P, B, L)),
                            op=OP.mult)
    alpha0 = gw.tile([P, B], F32, tag="sp_al0")
    nc.vector.tensor_tensor(alpha0[:], exw0[:], rs[:], op=OP.mult)
    wr = gw.tile([P, BL, D], F32, tag="sp_wr")
    nc.vector.tensor_tensor(wr[:], Gap[:, :, 0:D],
                            alpha[:].rearrange("p b l -> p (b l)")
                            .to_broadcast((P, BL, D)), op=OP.mult)
    if L > 1:
        msgs = gw.tile([P, B, D], F32, tag="sp_msgs")
        nc.vector.tensor_reduce(
            msgs[:], wr[:].rearrange("p (b l) f -> p b f l", b=B),
            axis=mybir.AxisListType.X, op=OP.add)
    else:
        msgs = wr
    t1 = gw.tile([P, B, D], F32, tag="sp_t1")
    nc.vector.tensor_tensor(t1[:], alpha0[:].to_broadcast((P, B, D)),
                            defrowv.to_broadcast((P, B, D)), op=OP.mult)
    if out_ap is None:
        msg = gw.tile([P, B, D], F32, tag="sp_msg")
        out_ap = msg[:]
    else:
        msg = None
    nc.vector.tensor_tensor(out_ap, msgs[:], t1[:], op=OP.add)
    return msg


def build(meta, repeat=1, stages="ducge"):
    """Build the SPMD Bass program (common across cores).

    stages: subset of 'd' (default writes), 'u' (l1 endpoint table),
    'c' (l1 conv -> tab rows), 'g' (l2 gather), 'e' (l2 emit+write)."""
    K = meta["K"]
    nblk1, nUt = meta["nblk1"], meta["nUt"]
    S1, groups1, L1 = meta["S1"], meta["groups1"], meta["L1"]
    S_g, L0, groups2b = meta["S_g"], meta["L0"], meta["groups2b"]
    ncompb, nblk2 = meta["ncompb"], meta["nblk2"]
    ROW_B2, REPL_LO = meta["ROW_B2"], meta["REPL_LO"]
    NPRM = 4 * D + 5
    nf1 = S1 + nblk1
    nf2 = S_g + L0 + 2 * (ncompb - 1)

    nc = bacc.Bacc("TRN2", target_bir_lowering=False, debug=False,
                   num_devices=NCORES)
    dt = nc.dram_tensor
    xranges = meta["xranges"]
    x_in = dt("x_in", [N, D], F32, kind="ExternalInput").ap()
    params_in = dt("params_in", [D, NPRM], F32, kind="ExternalInput").ap()
    b2row_in = dt("b2row_in", [1, D], F32, kind="ExternalInput").ap()
    uidx_in = dt("uidx_in", [P, 8 * nUt], I16, kind="ExternalInput").ap()
    l1_eidx_in = dt("l1_eidx_in", [P, 8 * S1], I16, kind="ExternalInput").ap()
    l1_f_in = dt("l1_f_in", [P, nf1], F32, kind="ExternalInput").ap()
    eidx2_in = dt("eidx2_in", [P, 8 * S_g], I16, kind="ExternalInput").ap()
    l2f_in = dt("l2f_in", [P, nf2], F32, kind="ExternalInput").ap()
    out_t = dt("out", [meta["npos"], D], F32, kind="ExternalOutput").ap()

    h1tab = dt("h1tab", [nUt * P, P], F32).ap()
    tab = dt("tab", [VTAB, P], F32).ap()

    with tile.TileContext(nc) as tc, ExitStack() as ctx:
        const = ctx.enter_context(tc.tile_pool(name="const", bufs=1))
        psc_ctx = tc.tile_pool(name="psc", bufs=1, space="PSUM")
        psc = psc_ctx.__enter__()

        ident = const.tile([P, P], F32)
        make_identity(nc, ident[:])

        # ---- parameters (one blob: W1|W1T|W2|W2T|av1|av2|b1col) ----
        prm = const.tile([D, NPRM], F32)
        nc.sync.dma_start(prm[:], params_in[:])
        W1s = prm[:, 0:D]
        W1Ts = prm[:, D:2 * D]
        W2s = prm[:, 2 * D:3 * D]
        W2Ts = prm[:, 3 * D:4 * D]
        av1s = prm[:, 4 * D:4 * D + 2]
        av2s = prm[:, 4 * D + 2:4 * D + 4]
        b1cols = prm[:, 4 * D + 4:4 * D + 5]
        b2rows = const.tile([1, D], F32)
        nc.sync.dma_start(b2rows[:], b2row_in[:])

        wt1_p = psc.tile([D, 2], F32, space="PSUM")
        nc.tensor.matmul(wt1_p[:], W1Ts, av1s, start=True, stop=True)
        wt2_p = psc.tile([D, 2], F32, space="PSUM")
        nc.tensor.matmul(wt2_p[:], W2Ts, av2s, start=True, stop=True)
        wt2s = const.tile([D, 2], F32)
        nc.vector.tensor_copy(wt2s[:], wt2_p[:])

        W1aug = const.tile([D, D + 2], F32)
        nc.vector.tensor_copy(W1aug[:, 0:D], W1s)
        nc.vector.tensor_copy(W1aug[:, D:D + 2], wt1_p[:])

        # SPEC2 [65, 66] = [[W2 | wt2s wt2d]; [b1@W2+b2 | b1.wt2s b1.wt2d]]
        SPEC = const.tile([D + 1, D + 2], F32)
        nc.vector.tensor_copy(SPEC[0:D, 0:D], W2s)
        nc.vector.tensor_copy(SPEC[0:D, D:D + 2], wt2s[:])
        b1w2_p = psc.tile([1, D], F32, space="PSUM")
        nc.tensor.matmul(b1w2_p[:], b1cols, W2s, start=True, stop=True)
        nc.vector.tensor_tensor(SPEC[D:D + 1, 0:D], b1w2_p[:], b2rows[:],
                                op=OP.add)
        b1w_p = psc.tile([1, 2], F32, space="PSUM")
        nc.tensor.matmul(b1w_p[:], b1cols, wt2s[:], start=True, stop=True)
        nc.vector.tensor_copy(SPEC[D:D + 1, D:D + 2], b1w_p[:])

        # block-diagonal pair matrices (two 64-wide tiles per PE pass)
        W1aug2 = const.tile([2 * D, 2 * (D + 2)], F32)
        nc.vector.memset(W1aug2[:], 0.0)
        nc.vector.tensor_copy(W1aug2[0:D, 0:D + 2], W1aug[:])
        nc.vector.tensor_copy(W1aug2[D:2 * D, D + 2:2 * (D + 2)], W1aug[:])
        W2D = const.tile([2 * D, 2 * (D + 2)], F32)
        nc.vector.memset(W2D[:], 0.0)
        nc.vector.tensor_copy(W2D[0:D, 0:D + 2], SPEC[0:D, :])
        nc.vector.tensor_copy(W2D[D:2 * D, D + 2:2 * (D + 2)], SPEC[0:D, :])

        # ---- constant table rows + replicated default row ----
        row0_s = const.tile([1, P], F32)
        nc.vector.memset(row0_s[:], 0.0)
        nc.vector.tensor_copy(row0_s[:, 0:D + 2], SPEC[D:D + 1, :])
        b2r_s = const.tile([1, P], F32)
        nc.vector.memset(b2r_s[:], 0.0)
        nc.vector.tensor_copy(b2r_s[:, 0:D], b2rows[:])
        ones_s = const.tile([1, P], F32)
        nc.vector.memset(ones_s[:], 1.0)
        repl_p = psc.tile([P, P], F32, space="PSUM")
        nc.tensor.matmul(repl_p[:], ones_s[:], row0_s[:], start=True, stop=True)
        repl_s = const.tile([P, P], F32)
        nc.vector.tensor_copy(repl_s[:], repl_p[:])
        defrowv = repl_s[:, 0:D].rearrange("p (k f) -> p k f", k=1)
        csrep = repl_s[:, D:D + 1]
        cdrep = repl_s[:, D + 1:D + 2]
        replv = repl_s[:].rearrange("p (k f) -> p k f", k=1)
        KB = 16                      # blocks per default-write chunk
        defbig = const.tile([P, KB * D], F32)
        nc.vector.tensor_copy(
            defbig[:].rearrange("p (k f) -> p k f", k=KB),
            defrowv.to_broadcast((P, KB, D)))

        nc.sync.dma_start(tab[0:1, :], row0_s[:])
        nc.sync.dma_start(tab[ROW_B2:ROW_B2 + 1, :], b2r_s[:])
        nc.sync.dma_start(
            tab[REPL_LO:REPL_LO + NREPL, :].rearrange("(k p) f -> p k f", p=P),
            replv.to_broadcast((P, NREPL // P, P)))

        # default-score constant e0c = lrelu(c_s + c_d) + BIG
        s0c = const.tile([P, 1], F32)
        nc.vector.tensor_tensor(s0c[:], csrep, cdrep, op=OP.add)
        u0c = const.tile([P, 1], F32)
        nc.vector.scalar_tensor_tensor(u0c[:], s0c[:], NEG_SLOPE, s0c[:],
                                       op0=OP.mult, op1=OP.max)
        e0c = const.tile([P, 1], F32)
        nc.vector.tensor_scalar_add(e0c[:], u0c[:], BIG)

        psc_ctx.__exit__(None, None, None)

        # ---- index tensors ----
        uidx_s = const.tile([P, 8 * nUt], I16)
        nc.sync.dma_start(uidx_s[:], uidx_in[:])
        l1_eidx_s = const.tile([P, 8 * S1], I16)
        nc.sync.dma_start(l1_eidx_s[:], l1_eidx_in[:])
        l1_f_s = const.tile([P, nf1], F32)
        nc.sync.dma_start(l1_f_s[:], l1_f_in[:])
        l1_mask_s = l1_f_s[:, 0:S1]
        l1_degpos_s = l1_f_s[:, S1:S1 + nblk1]
        eidx2_s = const.tile([P, 8 * S_g], I16)
        nc.sync.dma_start(eidx2_s[:], eidx2_in[:])
        l2f_s = const.tile([P, nf2], F32)
        nc.sync.dma_start(l2f_s[:], l2f_in[:])
        mask2_s = l2f_s[:, 0:S_g]
        wts0_s = l2f_s[:, S_g:S_g + L0]
        m0b_s = l2f_s[:, S_g + L0:S_g + L0 + (ncompb - 1)]
        w0b_s = l2f_s[:, S_g + L0 + (ncompb - 1):nf2]

        # persistent staging tiles (values rewritten every rep)
        h_all = const.tile([P, nUt, D + 2], F32)
        npr1 = nblk1 // 2
        mTs_c = const.tile([2 * D, max(npr1, 1), P], F32)
        mTs_s = const.tile([D, P], F32)
        repl2 = const.tile([P, 2 * (D + 2)], F32)
        nc.vector.tensor_copy(repl2[:, 0:D + 2], repl_s[:, 0:D + 2])
        nc.vector.tensor_copy(repl2[:, D + 2:2 * (D + 2)],
                              repl_s[:, 0:D + 2])
        row_all = const.tile([P, nblk1, D + 2], F32)
        msg1_all = const.tile([P, nblk1, D], F32)
        msgall = const.tile([P, ncompb, D], F32)

        ndefblk = nblk2 - ncompb
        outcmp = out_t[0:ncompb * P, :].rearrange("(p b) f -> p b f", b=ncompb)

        l1u = ctx.enter_context(tc.tile_pool(name="l1u", bufs=2))
        l1up = ctx.enter_context(tc.tile_pool(name="l1up", bufs=2,
                                              space="PSUM"))
        l1w = ctx.enter_context(tc.tile_pool(name="l1w", bufs=2))
        l1p = ctx.enter_context(tc.tile_pool(name="l1p", bufs=2, space="PSUM"))
        gw = ctx.enter_context(tc.tile_pool(name="gw", bufs=2))

        for _rep in range(repeat):
            # ---- default-region output: chunked contiguous writes ----
            if "d" in stages:
                b = 0
                while b < ndefblk:
                    nb = min(KB, ndefblk - b)
                    r0 = (ncompb + b) * P
                    nc.sync.dma_start(
                        out_t[r0:r0 + nb * P, :].rearrange(
                            "(p k) f -> p (k f)", k=nb),
                        defbig[:, 0:nb * D])
                    b += nb

            # ---- layer 1: h1 table for the U endpoint nodes ----
            if "u" not in stages:
                continue
            if True:
                xall = l1u.tile([P, nUt, D], F32, tag="xall")
                toff = 0
                for lo, nt in xranges:
                    hi = min(lo + (1 << 15), N)
                    nc.gpsimd.dma_gather(
                        xall[:, toff:toff + nt, :], x_in[lo:hi, :],
                        uidx_s[:, 8 * toff:8 * (toff + nt)],
                        nt * P, nt * P, D, single_packet=False)
                    toff += nt
                for t2 in range(nUt // 2):
                    xT_p = l1up.tile([P, P], F32, space="PSUM", tag="xT")
                    nc.tensor.transpose(
                        xT_p[:],
                        xall[:, 2 * t2:2 * t2 + 2, :]
                        .rearrange("p k f -> p (k f)"), ident[:])
                    xT_s = l1u.tile([P, P], F32, tag="xTs")
                    nc.vector.tensor_copy(xT_s[:], xT_p[:])
                    h_p = l1up.tile([P, 2 * (D + 2)], F32, space="PSUM",
                                    tag="h_p")
                    nc.tensor.matmul(h_p[:], xT_s[:], W1aug2[:], start=True,
                                     stop=True)
                    nc.scalar.copy(
                        h_all[:, 2 * t2:2 * t2 + 2, :]
                        .rearrange("p k f -> p (k f)"), h_p[:])
                if nUt % 2:
                    t = nUt - 1
                    xT_p1 = l1up.tile([D, P], F32, space="PSUM", tag="xT1")
                    nc.tensor.transpose(xT_p1[:], xall[:, t, :], ident[:])
                    xT_s1 = l1u.tile([D, P], F32, tag="xTs1")
                    nc.vector.tensor_copy(xT_s1[:], xT_p1[:])
                    h_p1 = l1up.tile([P, D + 2], F32, space="PSUM", tag="h_p1")
                    nc.tensor.matmul(h_p1[:], xT_s1[:], W1aug[:], start=True,
                                     stop=True)
                    nc.scalar.copy(h_all[:, t, :], h_p1[:])
                nc.sync.dma_start(
                    h1tab[:, 0:D + 2].rearrange("(k p) f -> p k f", p=P),
                    h_all[:])

            # ---- layer 1 conv -> write special table rows 1..K ----
            if "c" not in stages:
                continue
            if True:
                G1 = l1w.tile([P, S1, P], F32, tag="G1")
                nc.gpsimd.dma_gather(G1[:], h1tab[:, :], l1_eidx_s[:],
                                     S1 * P, S1 * P, P, single_packet=False)
                for g in groups1:
                    B, L, off = g["B"], g["L"], g["slot_off"]
                    Gap = G1[:, off:off + B * L, :]
                    adst1 = _extract_lastslot(nc, l1w, Gap, B, L, D + 1,
                                              "adst1")
                    _emit_group(
                        nc, l1w, Gap, l1_mask_s[:, off:off + B * L],
                        adst1[:], B, L,
                        degpos_ap=l1_degpos_s[:, g["b0"]:g["b0"] + B],
                        out_ap=msg1_all[:, g["b0"]:g["b0"] + B, :])
                for pr in range(nblk1 // 2):
                    mT_p = l1p.tile([P, P], F32, space="PSUM", tag="mT")
                    nc.tensor.transpose(
                        mT_p[:],
                        msg1_all[:, 2 * pr:2 * pr + 2, :]
                        .rearrange("p k f -> p (k f)"), ident[:])
                    nc.vector.tensor_copy(mTs_c[:, pr, :], mT_p[:])
                    row_p = l1p.tile([P, 2 * (D + 2)], F32, space="PSUM",
                                     tag="rowp")
                    nc.tensor.matmul(row_p[:], mTs_c[:, pr, :], W2D[:],
                                     start=True, stop=True)
                    nc.vector.tensor_tensor(
                        row_all[:, 2 * pr:2 * pr + 2, :]
                        .rearrange("p k f -> p (k f)"), row_p[:], repl2[:],
                        op=OP.add)
                if nblk1 % 2:
                    b = nblk1 - 1
                    mT_p1 = l1p.tile([D, P], F32, space="PSUM", tag="mT1")
                    nc.tensor.transpose(mT_p1[:], msg1_all[:, b, :], ident[:])
                    nc.vector.tensor_copy(mTs_s[:], mT_p1[:])
                    row_p1 = l1p.tile([P, D + 2], F32, space="PSUM",
                                      tag="rowp1")
                    nc.tensor.matmul(row_p1[:], mTs_s[:], SPEC[0:D, :],
                                     start=True, stop=True)
                    nc.vector.tensor_tensor(row_all[:, b, :], row_p1[:],
                                            repl2[:, 0:D + 2], op=OP.add)
                nfull = K // P
                if nfull:
                    nc.sync.dma_start(
                        tab[1:1 + nfull * P, 0:D + 2].rearrange(
                            "(k p) f -> p k f", p=P),
                        row_all[:, 0:nfull, :])
                rem = K - nfull * P
                if rem:
                    nc.sync.dma_start(tab[1 + nfull * P:1 + K, 0:D + 2],
                                      row_all[0:rem, nfull, :])

            # ---- layer 2 ----
            if "g" not in stages:
                continue
            if True:
                G = gw.tile([P, S_g, P], F32, tag="G")
                nc.gpsimd.dma_gather(G[:], tab[:, :], eidx2_s[:],
                                     S_g * P, S_g * P, P, single_packet=False)
                if "e" not in stages:
                    dum = gw.tile([P, P], F32, tag="dum")
                    nc.vector.tensor_copy(dum[:], G[:, 0, :])
                    continue
                # block 0: full grid
                Gap0 = G[:, 0:L0, :]
                adst0 = _extract_lastslot(nc, gw, Gap0, 1, L0, D + 1, "adst0")
                _emit_group(nc, gw, Gap0, mask2_s[:, 0:L0], adst0[:],
                            1, L0, wts_ap=wts0_s[:],
                            out_ap=msgall[:, 0:1, :])
                # blocks >= 1: special-only grids
                for g in groups2b:
                    B, L, off = g["B"], g["L"], g["slot_off"]
                    Gap = G[:, L0 + off:L0 + off + B * L, :]
                    _emit_sp_group(
                        nc, gw, Gap, mask2_s[:, L0 + off:L0 + off + B * L],
                        m0b_s[:, g["b0"] - 1:g["b0"] - 1 + B],
                        w0b_s[:, g["b0"] - 1:g["b0"] - 1 + B],
                        e0c[:], cdrep, defrowv, B, L,
                        out_ap=msgall[:, g["b0"]:g["b0"] + B, :])
                nc.sync.dma_start(outcmp, msgall[:])

    nc.compile()
    return nc


def make_in_maps(inputs, meta, l1, cores):
    x = np.ascontiguousarray(np.asarray(inputs["x"], dtype=np.float32))
    W1 = np.asarray(inputs["W1"], dtype=np.float32)
    W2 = np.asarray(inputs["W2"], dtype=np.float32)
    params = np.concatenate(
        [W1, np.ascontiguousarray(W1.T), W2, np.ascontiguousarray(W2.T),
         np.stack([np.asarray(inputs["a_src1"]),
                   np.asarray(inputs["a_dst1"])], axis=1),
         np.stack([np.asarray(inputs["a_src2"]),
                   np.asarray(inputs["a_dst2"])], axis=1),
         np.asarray(inputs["b1"]).reshape(D, 1)],
        axis=1).astype(np.float32)
    base = {
        "x_in": x,
        "params_in": np.ascontiguousarray(params),
        "b2row_in": np.asarray(inputs["b2"], dtype=np.float32).reshape(1, D),
        "uidx_in": l1["uidx16"],
        "l1_eidx_in": l1["l1_eidx"],
        "l1_f_in": l1["l1_f"],
    }
    in_maps = []
    for c in range(NCORES):
        m = dict(base)
        m["eidx2_in"] = cores[c]["eidx2"]
        m["l2f_in"] = cores[c]["l2f"]
        in_maps.append(m)
    return in_maps


def unshard_core(oc, order, ncompb):
    got = np.empty((NPC, D), np.float32)
    nposc = ncompb * P
    pos = np.arange(nposc)
    got[order[:nposc]] = oc[(pos % P) * ncompb + pos // P]
    got[order[nposc:NPC]] = oc[nposc:NPC]
    return got


def unshard(results, cores, meta):
    out = np.empty((N, D), np.float32)
    for c in range(NCORES):
        out[c * NPC:(c + 1) * NPC] = unshard_core(
            results[c]["out"], cores[c]["order"], meta["ncompb"])
    return out


def kernel(**inputs):
    meta, l1, cores = prep(inputs)
    nc = build(meta, repeat=1)
    in_maps = make_in_maps(inputs, meta, l1, cores)
    res = run_bass_kernel_spmd(nc, in_maps, core_ids=list(range(NCORES)))
    return unshard(res.results, cores, meta)


# revision 34
# speedup vs baseline: 2.4697x; 1.0454x over previous
"""GAT 2-layer encoder on 8 Trainium2 NeuronCores.

Reference computation: layer 1 = GAT conv over edge_index[:, :500] (weights W1),
layer 2 = GAT conv over edge_index[:, 500:] (weights W2).

Strategy (sparse-special):
  - Layer-1 output x1 differs from b1 only on the K<=500 distinct dsts of the
    first 500 edges ("specials").  In layer 2, h2[src] = x1[src]@W2 is the
    constant default row for every non-special src, so only edges whose src is
    special (~8k of 1.6M) carry information.  For a dst with no special
    in-edge, softmax over equal scores gives alpha = 1/deg for every in-edge,
    hence out = b1@W2 + b2 exactly (up to the 1e-16 eps), a CONSTANT row.
  - Device builds a (K+2)-row table in h2-space with b2 baked in:
    row r = [x1_r@W2 + b2 | asrc2_r | adst2_r], row 0 = default, row K+1 = b2
    (for deg-0 dsts), rows K+2.. = replicas of row 0 (spread gather load).
  - Sharding: dst-range partition of the 1.6M layer-2 edges across 8 cores (no
    collectives; layer 1 + table build replicated on every core, it is tiny).
  - Per core: dsts sorted so that special-adst / deg-0 dsts land in block 0
    (full slot grid: special slots + default slot + dst slot, all gathered),
    remaining computed blocks carry ONLY special-edge slots; their default
    in-edge mass and adst2 = c_d are handled with per-position scalars.  The
    ~90 all-default blocks are written with one broadcast DMA of the constant
    row.  Layer-2 table rows are pre-transformed by W2, so no matmul there.
"""

import sys

sys.path.insert(0, "/opt/trn_rl_repo")

from contextlib import ExitStack

import numpy as np

import concourse.bacc as bacc
import concourse.bass as bass
import concourse.mybir as mybir
import concourse.tile as tile
from concourse.bass_utils import run_bass_kernel_spmd
from concourse.masks import make_identity

F32 = mybir.dt.float32
I16 = mybir.dt.int16
I32 = mybir.dt.int32
AF = mybir.ActivationFunctionType
OP = mybir.AluOpType

N = 100000
D = 64
NCORES = 8
NPC = N // NCORES          # dst nodes per core
P = 128
NSPLIT = 500               # first 500 edges -> layer 1
NEG_SLOPE = 0.2
BIG = 200.0                # score shift so padded slots underflow exp to 0.0
VTAB = 1024                # gather table rows (specials + default replicas)
NREPL = 512                # default-row replicas written (one broadcast DMA)


def _wrap16(flat):
    """int16 stream [n] (n%16==0) -> dma_gather idx tile [128, n//16]."""
    w = flat.reshape(-1, 16).T
    return np.ascontiguousarray(np.tile(w, (8, 1)).astype(np.int16))


def _groups_of(Ls, b0=0):
    """Split the per-block padded-degree profile into equal-L runs."""
    groups = []
    off = 0
    b = 0
    while b < len(Ls):
        s = b
        while b < len(Ls) and Ls[b] == Ls[s]:
            b += 1
        groups.append({"b0": b0 + s, "B": b - s, "L": Ls[s], "slot_off": off})
        off += (b - s) * Ls[s]
    return groups


def prep(inputs):
    """Host-side index prep (pure index computation, no feature values)."""
    ei = np.asarray(inputs["edge_index"])
    src = ei[0].astype(np.int64)
    dst = ei[1].astype(np.int64)
    s1, d1 = src[:NSPLIT], dst[:NSPLIT]
    s2, d2 = src[NSPLIT:], dst[NSPLIT:]

    # ---- layer 1 structure ----
    specials, deg1 = np.unique(d1, return_counts=True)
    K = len(specials)
    order1 = np.argsort(-deg1, kind="stable")
    spec_by_pos = specials[order1]          # grid position q -> node, table row q+1
    rowmap = np.zeros(N, np.int16)
    rowmap[spec_by_pos] = np.arange(1, K + 1)
    nblk1 = (K + P - 1) // P
    npos1 = nblk1 * P

    U = np.unique(np.concatenate([s1, d1]))
    nU = len(U)
    # x-row gather in int16-addressable ranges of 32768 rows
    RSPAN = 1 << 15
    xranges = []          # (lo, ntiles)
    uidx16_parts = []
    uindex = np.zeros(N, np.int64)
    off = 0
    for lo in range(0, N, RSPAN):
        hi = min(lo + RSPAN, N)
        Ur = U[(U >= lo) & (U < hi)]
        if len(Ur) == 0:
            continue
        nt = (len(Ur) + P - 1) // P
        pad = np.full(nt * P, lo, np.int64)
        pad[:len(Ur)] = Ur
        uindex[Ur] = off * P + np.arange(len(Ur))
        uidx16_parts.append(_wrap16((pad - lo).astype(np.int16)))
        xranges.append((lo, nt))
        off += nt
    nUt = off
    uidx16 = np.concatenate(uidx16_parts, axis=1)

    # layer-1 slot grid: per block [special-edge slots | dst slot]
    rank1 = np.empty(K, np.int64)
    rank1[order1] = np.arange(K)
    d1pos = rank1[np.searchsorted(specials, d1)]
    deg1_sorted = np.zeros(npos1, np.int64)
    deg1_sorted[:K] = deg1[order1]
    L1sp = [max(int(deg1_sorted[b * P:(b + 1) * P].max()), 1)
            for b in range(nblk1)]
    L1 = [l + 1 for l in L1sp]
    S1 = int(sum(L1))
    slot_base1 = np.concatenate([[0], np.cumsum(L1)])[:-1]
    idx1 = np.zeros(S1 * P, np.int16)
    mask1 = np.zeros(S1 * P, np.float32)
    pe = np.argsort(d1pos, kind="stable")
    pos_s = d1pos[pe]
    val_s = uindex[s1[pe]].astype(np.int16)
    start_of_pos = np.searchsorted(pos_s, np.arange(npos1))
    kk = np.arange(len(pos_s)) - start_of_pos[pos_s]
    flat = (slot_base1[pos_s // P] + kk) * P + (pos_s % P)
    idx1[flat] = val_s
    mask1[flat] = 1.0
    # dst slots (last slot of each block)
    posn = np.arange(npos1)
    dv1 = np.zeros(npos1, np.int16)
    dv1[:K] = uindex[spec_by_pos]
    fdst = (slot_base1[posn // P] + np.asarray(L1)[posn // P] - 1) * P + posn % P
    idx1[fdst] = dv1
    dp1 = np.zeros(npos1, np.float32)
    dp1[:K] = 1.0
    l1_eidx = _wrap16(idx1)
    l1_f = np.concatenate(
        [np.ascontiguousarray(mask1.reshape(S1, P).T),
         np.ascontiguousarray(dp1.reshape(nblk1, P).T)], axis=1)
    groups1 = _groups_of(L1)

    # ---- layer 2 structure (sparse-special grid) ----
    npos = ((NPC + P - 1) // P) * P
    nblk2 = npos // P
    ROW_B2 = K + 1
    REPL_LO = K + 2
    REPL_HI = REPL_LO + NREPL
    assert REPL_HI <= VTAB
    percore = []
    for c in range(NCORES):
        sel = (d2 >= c * NPC) & (d2 < (c + 1) * NPC)
        dl = d2[sel] - c * NPC
        sl = s2[sel]
        deg = np.bincount(dl, minlength=NPC)
        spr_all = rowmap[sl]
        m = spr_all > 0
        spd = dl[m]
        spr = spr_all[m]
        deg_sp = np.bincount(spd, minlength=NPC)
        ndef = deg - deg_sp
        spadst = rowmap[c * NPC:(c + 1) * NPC] > 0
        front = spadst | (deg == 0)
        assert int(front.sum()) <= P
        key = front.astype(np.int64) * (1 << 20) + deg_sp
        order = np.argsort(-key, kind="stable")
        ncomp = int((key > 0).sum())
        percore.append(dict(deg=deg, deg_sp=deg_sp, ndef=ndef, spd=spd,
                            spr=spr, order=order, ncomp=ncomp))
    ncompb = max(1, max((pc["ncomp"] + P - 1) // P for pc in percore))
    assert ncompb * P <= NPC

    def blkmax(pc, b):
        return int(pc["deg_sp"][pc["order"][b * P:(b + 1) * P]].max())

    L0 = max(blkmax(pc, 0) for pc in percore) + 2   # +default +dst slot
    Lb = [max(max(blkmax(pc, b) for pc in percore), 1)
          for b in range(1, ncompb)]
    S_g = L0 + int(sum(Lb))
    slot_base = np.zeros(ncompb, np.int64)                 # per-block slot base
    if ncompb > 1:
        slot_base[1:] = L0 + np.concatenate([[0], np.cumsum(Lb)[:-1]])
    cap = np.asarray([L0 - 2] + Lb)                        # special capacity
    groups2b = _groups_of(Lb, b0=1)

    cores = []
    nposc = ncompb * P
    for c, pc in enumerate(percore):
        deg, deg_sp, ndef = pc["deg"], pc["deg_sp"], pc["ndef"]
        spd, spr, order = pc["spd"], pc["spr"], pc["order"]
        rng = np.random.default_rng(1000 + c)
        idxflat = rng.integers(REPL_LO, REPL_HI, S_g * P).astype(np.int16)
        maskflat = np.zeros(S_g * P, np.float32)
        wts0 = np.zeros(L0 * P, np.float32)
        rank = np.empty(NPC, np.int64)
        rank[order] = np.arange(NPC)
        # special-edge slots
        pos = rank[spd]
        pe = np.argsort(pos, kind="stable")
        pos_s = pos[pe]
        val_s = spr[pe].astype(np.int16)
        assert pos_s.size == 0 or pos_s.max() < nposc
        start_of_pos = np.searchsorted(pos_s, np.arange(nposc))
        kk = np.arange(len(pos_s)) - start_of_pos[pos_s]
        assert np.all(kk < cap[pos_s // P])
        flat = (slot_base[pos_s // P] + kk) * P + (pos_s % P)
        idxflat[flat] = val_s
        maskflat[flat] = 1.0
        # block 0: default + dst slots
        p0 = np.arange(P)
        nodes0 = order[p0]
        nd0 = ndef[nodes0].astype(np.float32)
        is00 = deg[nodes0] == 0
        fd = (L0 - 2) * P + p0
        ft = (L0 - 1) * P + p0
        maskflat[fd] = ((nd0 > 0) | is00).astype(np.float32)
        wts0[0:(L0 - 2) * P] = maskflat[0:(L0 - 2) * P]
        wts0[fd] = np.where(is00, 1.0, nd0)
        idxflat[fd] = np.where(is00, np.int16(ROW_B2), idxflat[fd])
        rm0 = rowmap[c * NPC + nodes0]
        h0 = rm0 > 0
        idxflat[ft[h0]] = rm0[h0]
        # blocks >= 1 scalar fields
        posn = np.arange(P, nposc)
        nodesb = order[posn]
        assert np.all(deg[nodesb] > 0)
        m0b = (ndef[nodesb] > 0).astype(np.float32)
        w0b = ndef[nodesb].astype(np.float32)
        m0b_t = np.ascontiguousarray(m0b.reshape(ncompb - 1, P).T) \
            if ncompb > 1 else np.zeros((P, 0), np.float32)
        w0b_t = np.ascontiguousarray(w0b.reshape(ncompb - 1, P).T) \
            if ncompb > 1 else np.zeros((P, 0), np.float32)
        l2f = np.concatenate(
            [np.ascontiguousarray(maskflat.reshape(S_g, P).T),
             np.ascontiguousarray(wts0.reshape(L0, P).T),
             m0b_t, w0b_t], axis=1)
        cores.append({"eidx2": _wrap16(idxflat), "l2f": l2f, "order": order})

    meta = {
        "K": K, "nblk1": nblk1, "nU": nU, "nUt": nUt, "xranges": xranges,
        "L1": L1, "groups1": groups1, "S1": S1,
        "L0": L0, "Lb": Lb, "S_g": S_g, "groups2b": groups2b,
        "ncompb": ncompb, "nblk2": nblk2, "npos": npos,
        "ROW_B2": ROW_B2, "REPL_LO": REPL_LO,
    }
    l1 = {"uidx16": uidx16, "l1_eidx": l1_eidx, "l1_f": l1_f}
    return meta, l1, cores


def _extract_lastslot(nc, gw, Gap, B, L, col, tag):
    """[P, B] tile holding Gap[:, b*L + L-1, col] per block b."""
    t = gw.tile([P, B], F32, tag=tag)
    nc.scalar.activation(
        t[:],
        Gap[:, :, col:col + 1]
        .rearrange("p (b l) o -> p b (l o)", l=L)[:, :, L - 1:L]
        .rearrange("p b o -> p (b o)"),
        AF.Identity)
    return t


def _emit_group(nc, gw, Gap, mask_ap, adst_ap, B, L, wts_ap=None,
                degpos_ap=None, out_ap=None):
    """Segment softmax + weighted sum for B blocks of equal padded degree L.

    Gap: AP view [128, B*L, 128] of the gathered rows (slot-flat).
    Returns msg tile [128, B, 64]."""
    BL = B * L
    asrc = Gap[:, :, 64:65].rearrange("p s o -> p (s o)")        # [128, BL]
    s_t = gw.tile([P, B, L], F32, tag="s_t")
    nc.vector.tensor_tensor(s_t[:], asrc, adst_ap.to_broadcast((P, B, L)),
                            op=OP.add)
    u_t = gw.tile([P, B, L], F32, tag="u_t")
    nc.vector.scalar_tensor_tensor(u_t[:], s_t[:], NEG_SLOPE, s_t[:],
                                   op0=OP.mult, op1=OP.max)
    e2_t = gw.tile([P, B, L], F32, tag="e2_t")
    nc.vector.scalar_tensor_tensor(e2_t[:], u_t[:], BIG, mask_ap,
                                   op0=OP.add, op1=OP.mult)
    mneg = gw.tile([P, B], F32, tag="mneg")
    nc.vector.tensor_reduce(mneg[:], e2_t[:], axis=mybir.AxisListType.X,
                            op=OP.max, negate=True)
    d_t = gw.tile([P, B, L], F32, tag="d_t")
    nc.vector.tensor_tensor(d_t[:], e2_t[:], mneg[:].to_broadcast((P, B, L)),
                            op=OP.add)
    ex_t = gw.tile([P, B, L], F32, tag="ex_t")
    nc.scalar.activation(ex_t[:], d_t[:], AF.Exp)
    if wts_ap is not None:
        exw_t = gw.tile([P, B, L], F32, tag="exw_t")
        nc.vector.tensor_tensor(exw_t[:], ex_t[:], wts_ap, op=OP.mult)
    else:
        exw_t = ex_t
    ssum = gw.tile([P, B], F32, tag="ssum")
    nc.vector.tensor_reduce(ssum[:], exw_t[:], axis=mybir.AxisListType.X,
                            op=OP.add)
    rs = gw.tile([P, B], F32, tag="rs")
    nc.vector.reciprocal(rs[:], ssum[:])
    if degpos_ap is not None:
        rsd = gw.tile([P, B], F32, tag="rsd")
        nc.vector.tensor_tensor(rsd[:], rs[:], degpos_ap, op=OP.mult)
    else:
        rsd = rs
    alpha = gw.tile([P, B, L], F32, tag="alpha")
    nc.vector.tensor_tensor(alpha[:], exw_t[:], rsd[:].to_broadcast((P, B, L)),
                            op=OP.mult)
    wr = gw.tile([P, BL, D], F32, tag="wr")
    nc.vector.tensor_tensor(wr[:], Gap[:, :, 0:D],
                            alpha[:].rearrange("p b l -> p (b l)")
                            .to_broadcast((P, BL, D)), op=OP.mult)
    if out_ap is None:
        msg = gw.tile([P, B, D], F32, tag="msg")
        out_ap = msg[:]
    else:
        msg = None
    nc.vector.tensor_reduce(out_ap,
                            wr[:].rearrange("p (b l) f -> p b f l", b=B),
                            axis=mybir.AxisListType.X, op=OP.add)
    return msg


def _emit_sp_group(nc, gw, Gap, mask_ap, m0b_ap, w0b_ap, e0c, cdrep, defrowv,
                   B, L, out_ap=None):
    """Blocks with only special-edge slots: default-edge mass via scalars.

    Gap [128, B*L, 128]; adst = c_d (non-special dsts); e0c [P,1] = default
    score lrelu(c_s+c_d)+BIG; m0b/w0b [P,B] = (ndef>0) and ndef.
    Returns msg tile [128, B, 64] (includes the default-row contribution)."""
    BL = B * L
    asrc = Gap[:, :, 64:65].rearrange("p s o -> p (s o)")
    s_t = gw.tile([P, B, L], F32, tag="sp_s")
    nc.vector.tensor_tensor(s_t[:], asrc, cdrep.to_broadcast((P, B, L)),
                            op=OP.add)
    u_t = gw.tile([P, B, L], F32, tag="sp_u")
    nc.vector.scalar_tensor_tensor(u_t[:], s_t[:], NEG_SLOPE, s_t[:],
                                   op0=OP.mult, op1=OP.max)
    e2_t = gw.tile([P, B, L], F32, tag="sp_e2")
    nc.vector.scalar_tensor_tensor(e2_t[:], u_t[:], BIG, mask_ap,
                                   op0=OP.add, op1=OP.mult)
    e0e = gw.tile([P, B], F32, tag="sp_e0")
    nc.vector.tensor_tensor(e0e[:], m0b_ap, e0c.to_broadcast((P, B)),
                            op=OP.mult)
    m_t = gw.tile([P, B], F32, tag="sp_m")
    if L > 1:
        msp = gw.tile([P, B], F32, tag="sp_msp")
        nc.vector.tensor_reduce(msp[:], e2_t[:], axis=mybir.AxisListType.X,
                                op=OP.max)
        nc.vector.tensor_tensor(m_t[:], msp[:], e0e[:], op=OP.max)
    else:
        nc.vector.tensor_tensor(
            m_t[:], e2_t[:].rearrange("p b l -> p (b l)"), e0e[:], op=OP.max)
    d_t = gw.tile([P, B, L], F32, tag="sp_d")
    nc.vector.tensor_tensor(d_t[:], e2_t[:], m_t[:].to_broadcast((P, B, L)),
                            op=OP.subtract)
    ex_t = gw.tile([P, B, L], F32, tag="sp_ex")
    nc.scalar.activation(ex_t[:], d_t[:], AF.Exp)
    d0 = gw.tile([P, B], F32, tag="sp_d0")
    nc.vector.tensor_tensor(d0[:], e0e[:], m_t[:], op=OP.subtract)
    ex0 = gw.tile([P, B], F32, tag="sp_ex0")
    nc.scalar.activation(ex0[:], d0[:], AF.Exp)
    exw0 = gw.tile([P, B], F32, tag="sp_exw0")
    nc.vector.tensor_tensor(exw0[:], ex0[:], w0b_ap, op=OP.mult)
    ssum = gw.tile([P, B], F32, tag="sp_ssum")
    if L > 1:
        ssp = gw.tile([P, B], F32, tag="sp_ssp")
        nc.vector.tensor_reduce(ssp[:], ex_t[:], axis=mybir.AxisListType.X,
                                op=OP.add)
        nc.vector.tensor_tensor(ssum[:], ssp[:], exw0[:], op=OP.add)
    else:
        nc.vector.tensor_tensor(
            ssum[:], ex_t[:].rearrange("p b l -> p (b l)"), exw0[:], op=OP.add)
    rs = gw.tile([P, B], F32, tag="sp_rs")
    nc.vector.reciprocal(rs[:], ssum[:])
    alpha = gw.tile([P, B, L], F32, tag="sp_al")
    nc.vector.tensor_tensor(alpha[:], ex_t[:], rs[:].to_broadcast((P, B, L)),
                            op=OP.mult)
    alpha0 = gw.tile([P, B], F32, tag="sp_al0")
    nc.vector.tensor_tensor(alpha0[:], exw0[:], rs[:], op=OP.mult)
    wr = gw.tile([P, BL, D], F32, tag="sp_wr")
    nc.vector.tensor_tensor(wr[:], Gap[:, :, 0:D],
                            alpha[:].rearrange("p b l -> p (b l)")
                            .to_broadcast((P, BL, D)), op=OP.mult)
    if L > 1:
        msgs = gw.tile([P, B, D], F32, tag="sp_msgs")
        nc.vector.tensor_reduce(
            msgs[:], wr[:].rearrange("p (b l) f -> p b f l", b=B),
            axis=mybir.AxisListType.X, op=OP.add)
    else:
        msgs = wr
    t1 = gw.tile([P, B, D], F32, tag="sp_t1")
    nc.vector.tensor_tensor(t1[:], alpha0[:].to_broadcast((P, B, D)),
                            defrowv.to_broadcast((P, B, D)), op=OP.mult)
    if out_ap is None:
        msg = gw.tile([P, B, D], F32, tag="sp_msg")
        out_ap = msg[:]
    else:
        msg = None
    nc.vector.tensor_tensor(out_ap, msgs[:], t1[:], op=OP.add)
    return msg


def build(meta, repeat=1, stages="ducge"):
    """Build the SPMD Bass program (common across cores).

    stages: subset of 'd' (default writes), 'u' (l1 endpoint table),
    'c' (l1 conv -> tab rows), 'g' (l2 gather), 'e' (l2 emit+write)."""
    K = meta["K"]
    nblk1, nUt = meta["nblk1"], meta["nUt"]
    S1, groups1, L1 = meta["S1"], meta["groups1"], meta["L1"]
    S_g, L0, groups2b = meta["S_g"], meta["L0"], meta["groups2b"]
    ncompb, nblk2 = meta["ncompb"], meta["nblk2"]
    ROW_B2, REPL_LO = meta["ROW_B2"], meta["REPL_LO"]
    NPRM = 4 * D + 5
    nf1 = S1 + nblk1
    nf2 = S_g + L0 + 2 * (ncompb - 1)

    nc = bacc.Bacc("TRN2", target_bir_lowering=False, debug=False,
                   num_devices=NCORES)
    dt = nc.dram_tensor
    xranges = meta["xranges"]
    x_in = dt("x_in", [N, D], F32, kind="ExternalInput").ap()
    params_in = dt("params_in", [D, NPRM], F32, kind="ExternalInput").ap()
    b2row_in = dt("b2row_in", [1, D], F32, kind="ExternalInput").ap()
    uidx_in = dt("uidx_in", [P, 8 * nUt], I16, kind="ExternalInput").ap()
    l1_eidx_in = dt("l1_eidx_in", [P, 8 * S1], I16, kind="ExternalInput").ap()
    l1_f_in = dt("l1_f_in", [P, nf1], F32, kind="ExternalInput").ap()
    eidx2_in = dt("eidx2_in", [P, 8 * S_g], I16, kind="ExternalInput").ap()
    l2f_in = dt("l2f_in", [P, nf2], F32, kind="ExternalInput").ap()
    out_t = dt("out", [meta["npos"], D], F32, kind="ExternalOutput").ap()

    h1tab = dt("h1tab", [nUt * P, P], F32).ap()
    tab = dt("tab", [VTAB, P], F32).ap()

    with tile.TileContext(nc) as tc, ExitStack() as ctx:
        const = ctx.enter_context(tc.tile_pool(name="const", bufs=1))
        psc_ctx = tc.tile_pool(name="psc", bufs=1, space="PSUM")
        psc = psc_ctx.__enter__()

        ident = const.tile([P, P], F32)
        make_identity(nc, ident[:])

        # ---- parameters (one blob: W1|W1T|W2|W2T|av1|av2|b1col) ----
        prm = const.tile([D, NPRM], F32)
        nc.sync.dma_start(prm[:], params_in[:])
        W1s = prm[:, 0:D]
        W1Ts = prm[:, D:2 * D]
        W2s = prm[:, 2 * D:3 * D]
        W2Ts = prm[:, 3 * D:4 * D]
        av1s = prm[:, 4 * D:4 * D + 2]
        av2s = prm[:, 4 * D + 2:4 * D + 4]
        b1cols = prm[:, 4 * D + 4:4 * D + 5]
        b2rows = const.tile([1, D], F32)
        nc.sync.dma_start(b2rows[:], b2row_in[:])

        wt1_p = psc.tile([D, 2], F32, space="PSUM")
        nc.tensor.matmul(wt1_p[:], W1Ts, av1s, start=True, stop=True)
        wt2_p = psc.tile([D, 2], F32, space="PSUM")
        nc.tensor.matmul(wt2_p[:], W2Ts, av2s, start=True, stop=True)
        wt2s = const.tile([D, 2], F32)
        nc.vector.tensor_copy(wt2s[:], wt2_p[:])

        W1aug = const.tile([D, D + 2], F32)
        nc.vector.tensor_copy(W1aug[:, 0:D], W1s)
        nc.vector.tensor_copy(W1aug[:, D:D + 2], wt1_p[:])

        # SPEC2 [65, 66] = [[W2 | wt2s wt2d]; [b1@W2+b2 | b1.wt2s b1.wt2d]]
        SPEC = const.tile([D + 1, D + 2], F32)
        nc.vector.tensor_copy(SPEC[0:D, 0:D], W2s)
        nc.vector.tensor_copy(SPEC[0:D, D:D + 2], wt2s[:])
        b1w2_p = psc.tile([1, D], F32, space="PSUM")
        nc.tensor.matmul(b1w2_p[:], b1cols, W2s, start=True, stop=True)
        nc.vector.tensor_tensor(SPEC[D:D + 1, 0:D], b1w2_p[:], b2rows[:],
                                op=OP.add)
        b1w_p = psc.tile([1, 2], F32, space="PSUM")
        nc.tensor.matmul(b1w_p[:], b1cols, wt2s[:], start=True, stop=True)
        nc.vector.tensor_copy(SPEC[D:D + 1, D:D + 2], b1w_p[:])

        # block-diagonal pair matrices (two 64-wide tiles per PE pass)
        W1aug2 = const.tile([2 * D, 2 * (D + 2)], F32)
        nc.vector.memset(W1aug2[:], 0.0)
        nc.vector.tensor_copy(W1aug2[0:D, 0:D + 2], W1aug[:])
        nc.vector.tensor_copy(W1aug2[D:2 * D, D + 2:2 * (D + 2)], W1aug[:])
        W2D = const.tile([2 * D, 2 * (D + 2)], F32)
        nc.vector.memset(W2D[:], 0.0)
        nc.vector.tensor_copy(W2D[0:D, 0:D + 2], SPEC[0:D, :])
        nc.vector.tensor_copy(W2D[D:2 * D, D + 2:2 * (D + 2)], SPEC[0:D, :])

        # ---- constant table rows + replicated default row ----
        row0_s = const.tile([1, P], F32)
        nc.vector.memset(row0_s[:], 0.0)
        nc.vector.tensor_copy(row0_s[:, 0:D + 2], SPEC[D:D + 1, :])
        b2r_s = const.tile([1, P], F32)
        nc.vector.memset(b2r_s[:], 0.0)
        nc.vector.tensor_copy(b2r_s[:, 0:D], b2rows[:])
        ones_s = const.tile([1, P], F32)
        nc.vector.memset(ones_s[:], 1.0)
        repl_p = psc.tile([P, P], F32, space="PSUM")
        nc.tensor.matmul(repl_p[:], ones_s[:], row0_s[:], start=True, stop=True)
        repl_s = const.tile([P, P], F32)
        nc.vector.tensor_copy(repl_s[:], repl_p[:])
        defrowv = repl_s[:, 0:D].rearrange("p (k f) -> p k f", k=1)
        csrep = repl_s[:, D:D + 1]
        cdrep = repl_s[:, D + 1:D + 2]
        replv = repl_s[:].rearrange("p (k f) -> p k f", k=1)
        KB = 16                      # blocks per default-write chunk
        defbig = const.tile([P, KB * D], F32)
        nc.vector.tensor_copy(
            defbig[:].rearrange("p (k f) -> p k f", k=KB),
            defrowv.to_broadcast((P, KB, D)))

        nc.sync.dma_start(tab[0:1, :], row0_s[:])
        nc.sync.dma_start(tab[ROW_B2:ROW_B2 + 1, :], b2r_s[:])
        nc.sync.dma_start(
            tab[REPL_LO:REPL_LO + NREPL, :].rearrange("(k p) f -> p k f", p=P),
            replv.to_broadcast((P, NREPL // P, P)))

        # default-score constant e0c = lrelu(c_s + c_d) + BIG
        s0c = const.tile([P, 1], F32)
        nc.vector.tensor_tensor(s0c[:], csrep, cdrep, op=OP.add)
        u0c = const.tile([P, 1], F32)
        nc.vector.scalar_tensor_tensor(u0c[:], s0c[:], NEG_SLOPE, s0c[:],
                                       op0=OP.mult, op1=OP.max)
        e0c = const.tile([P, 1], F32)
        nc.vector.tensor_scalar_add(e0c[:], u0c[:], BIG)

        psc_ctx.__exit__(None, None, None)

        # ---- index tensors ----
        uidx_s = const.tile([P, 8 * nUt], I16)
        nc.sync.dma_start(uidx_s[:], uidx_in[:])
        l1_eidx_s = const.tile([P, 8 * S1], I16)
        nc.sync.dma_start(l1_eidx_s[:], l1_eidx_in[:])
        l1_f_s = const.tile([P, nf1], F32)
        nc.sync.dma_start(l1_f_s[:], l1_f_in[:])
        l1_mask_s = l1_f_s[:, 0:S1]
        l1_degpos_s = l1_f_s[:, S1:S1 + nblk1]
        eidx2_s = const.tile([P, 8 * S_g], I16)
        nc.sync.dma_start(eidx2_s[:], eidx2_in[:])
        l2f_s = const.tile([P, nf2], F32)
        nc.sync.dma_start(l2f_s[:], l2f_in[:])
        mask2_s = l2f_s[:, 0:S_g]
        wts0_s = l2f_s[:, S_g:S_g + L0]
        m0b_s = l2f_s[:, S_g + L0:S_g + L0 + (ncompb - 1)]
        w0b_s = l2f_s[:, S_g + L0 + (ncompb - 1):nf2]

        # persistent staging tiles (values rewritten every rep)
        h_all = const.tile([P, nUt, D + 2], F32)
        npr1 = nblk1 // 2
        mTs_c = const.tile([2 * D, max(npr1, 1), P], F32)
        mTs_s = const.tile([D, P], F32)
        repl2 = const.tile([P, 2 * (D + 2)], F32)
        nc.vector.tensor_copy(repl2[:, 0:D + 2], repl_s[:, 0:D + 2])
        nc.vector.tensor_copy(repl2[:, D + 2:2 * (D + 2)],
                              repl_s[:, 0:D + 2])
        row_all = const.tile([P, nblk1, D + 2], F32)
        msg1_all = const.tile([P, nblk1, D], F32)
        msgall = const.tile([P, ncompb, D], F32)

        ndefblk = nblk2 - ncompb
        outcmp = out_t[0:ncompb * P, :].rearrange("(p b) f -> p b f", b=ncompb)

        l1u = ctx.enter_context(tc.tile_pool(name="l1u", bufs=3))
        l1up = ctx.enter_context(tc.tile_pool(name="l1up", bufs=2,
                                              space="PSUM"))
        l1w = ctx.enter_context(tc.tile_pool(name="l1w", bufs=3))
        l1p = ctx.enter_context(tc.tile_pool(name="l1p", bufs=2, space="PSUM"))
        gw = ctx.enter_context(tc.tile_pool(name="gw", bufs=3))

        for _rep in range(repeat):
            # ---- default-region output: chunked contiguous writes ----
            if "d" in stages:
                b = 0
                while b < ndefblk:
                    nb = min(KB, ndefblk - b)
                    r0 = (ncompb + b) * P
                    nc.sync.dma_start(
                        out_t[r0:r0 + nb * P, :].rearrange(
                            "(p k) f -> p (k f)", k=nb),
                        defbig[:, 0:nb * D])
                    b += nb

            # ---- layer 1: h1 table for the U endpoint nodes ----
            if "u" not in stages:
                continue
            if True:
                xall = l1u.tile([P, nUt, D], F32, tag="xall")
                toff = 0
                for lo, nt in xranges:
                    hi = min(lo + (1 << 15), N)
                    nc.gpsimd.dma_gather(
                        xall[:, toff:toff + nt, :], x_in[lo:hi, :],
                        uidx_s[:, 8 * toff:8 * (toff + nt)],
                        nt * P, nt * P, D, single_packet=False)
                    toff += nt
                for t2 in range(nUt // 2):
                    xT_p = l1up.tile([P, P], F32, space="PSUM", tag="xT")
                    nc.tensor.transpose(
                        xT_p[:],
                        xall[:, 2 * t2:2 * t2 + 2, :]
                        .rearrange("p k f -> p (k f)"), ident[:])
                    xT_s = l1u.tile([P, P], F32, tag="xTs")
                    nc.vector.tensor_copy(xT_s[:], xT_p[:])
                    h_p = l1up.tile([P, 2 * (D + 2)], F32, space="PSUM",
                                    tag="h_p")
                    nc.tensor.matmul(h_p[:], xT_s[:], W1aug2[:], start=True,
                                     stop=True)
                    nc.scalar.copy(
                        h_all[:, 2 * t2:2 * t2 + 2, :]
                        .rearrange("p k f -> p (k f)"), h_p[:])
                if nUt % 2:
                    t = nUt - 1
                    xT_p1 = l1up.tile([D, P], F32, space="PSUM", tag="xT1")
                    nc.tensor.transpose(xT_p1[:], xall[:, t, :], ident[:])
                    xT_s1 = l1u.tile([D, P], F32, tag="xTs1")
                    nc.vector.tensor_copy(xT_s1[:], xT_p1[:])
                    h_p1 = l1up.tile([P, D + 2], F32, space="PSUM", tag="h_p1")
                    nc.tensor.matmul(h_p1[:], xT_s1[:], W1aug[:], start=True,
                                     stop=True)
                    nc.scalar.copy(h_all[:, t, :], h_p1[:])
                nc.sync.dma_start(
                    h1tab[:, 0:D + 2].rearrange("(k p) f -> p k f", p=P),
                    h_all[:])

            # ---- layer 1 conv -> write special table rows 1..K ----
            if "c" not in stages:
                continue
            if True:
                G1 = l1w.tile([P, S1, P], F32, tag="G1")
                nc.gpsimd.dma_gather(G1[:], h1tab[:, :], l1_eidx_s[:],
                                     S1 * P, S1 * P, P, single_packet=False)
                for g in groups1:
                    B, L, off = g["B"], g["L"], g["slot_off"]
                    Gap = G1[:, off:off + B * L, :]
                    adst1 = _extract_lastslot(nc, l1w, Gap, B, L, D + 1,
                                              "adst1")
                    _emit_group(
                        nc, l1w, Gap, l1_mask_s[:, off:off + B * L],
                        adst1[:], B, L,
                        degpos_ap=l1_degpos_s[:, g["b0"]:g["b0"] + B],
                        out_ap=msg1_all[:, g["b0"]:g["b0"] + B, :])
                for pr in range(nblk1 // 2):
                    mT_p = l1p.tile([P, P], F32, space="PSUM", tag="mT")
                    nc.tensor.transpose(
                        mT_p[:],
                        msg1_all[:, 2 * pr:2 * pr + 2, :]
                        .rearrange("p k f -> p (k f)"), ident[:])
                    nc.vector.tensor_copy(mTs_c[:, pr, :], mT_p[:])
                    row_p = l1p.tile([P, 2 * (D + 2)], F32, space="PSUM",
                                     tag="rowp")
                    nc.tensor.matmul(row_p[:], mTs_c[:, pr, :], W2D[:],
                                     start=True, stop=True)
                    nc.vector.tensor_tensor(
                        row_all[:, 2 * pr:2 * pr + 2, :]
                        .rearrange("p k f -> p (k f)"), row_p[:], repl2[:],
                        op=OP.add)
                if nblk1 % 2:
                    b = nblk1 - 1
                    mT_p1 = l1p.tile([D, P], F32, space="PSUM", tag="mT1")
                    nc.tensor.transpose(mT_p1[:], msg1_all[:, b, :], ident[:])
                    nc.vector.tensor_copy(mTs_s[:], mT_p1[:])
                    row_p1 = l1p.tile([P, D + 2], F32, space="PSUM",
                                      tag="rowp1")
                    nc.tensor.matmul(row_p1[:], mTs_s[:], SPEC[0:D, :],
                                     start=True, stop=True)
                    nc.vector.tensor_tensor(row_all[:, b, :], row_p1[:],
                                            repl2[:, 0:D + 2], op=OP.add)
                nfull = K // P
                if nfull:
                    nc.sync.dma_start(
                        tab[1:1 + nfull * P, 0:D + 2].rearrange(
                            "(k p) f -> p k f", p=P),
                        row_all[:, 0:nfull, :])
                rem = K - nfull * P
                if rem:
                    nc.sync.dma_start(tab[1 + nfull * P:1 + K, 0:D + 2],
                                      row_all[0:rem, nfull, :])

            # ---- layer 2 ----
            if "g" not in stages:
                continue
            if True:
                G = gw.tile([P, S_g, P], F32, tag="G")
                nc.gpsimd.dma_gather(G[:], tab[:, :], eidx2_s[:],
                                     S_g * P, S_g * P, P, single_packet=False)
                if "e" not in stages:
                    dum = gw.tile([P, P], F32, tag="dum")
                    nc.vector.tensor_copy(dum[:], G[:, 0, :])
                    continue
                # block 0: full grid
                Gap0 = G[:, 0:L0, :]
                adst0 = _extract_lastslot(nc, gw, Gap0, 1, L0, D + 1, "adst0")
                _emit_group(nc, gw, Gap0, mask2_s[:, 0:L0], adst0[:],
                            1, L0, wts_ap=wts0_s[:],
                            out_ap=msgall[:, 0:1, :])
                # blocks >= 1: special-only grids
                for g in groups2b:
                    B, L, off = g["B"], g["L"], g["slot_off"]
                    Gap = G[:, L0 + off:L0 + off + B * L, :]
                    _emit_sp_group(
                        nc, gw, Gap, mask2_s[:, L0 + off:L0 + off + B * L],
                        m0b_s[:, g["b0"] - 1:g["b0"] - 1 + B],
                        w0b_s[:, g["b0"] - 1:g["b0"] - 1 + B],
                        e0c[:], cdrep, defrowv, B, L,
                        out_ap=msgall[:, g["b0"]:g["b0"] + B, :])
                nc.sync.dma_start(outcmp, msgall[:])

    nc.compile()
    return nc


def make_in_maps(inputs, meta, l1, cores):
    x = np.ascontiguousarray(np.asarray(inputs["x"], dtype=np.float32))
    W1 = np.asarray(inputs["W1"], dtype=np.float32)
    W2 = np.asarray(inputs["W2"], dtype=np.float32)
    params = np.concatenate(
        [W1, np.ascontiguousarray(W1.T), W2, np.ascontiguousarray(W2.T),
         np.stack([np.asarray(inputs["a_src1"]),
                   np.asarray(inputs["a_dst1"])], axis=1),
         np.stack([np.asarray(inputs["a_src2"]),
                   np.asarray(inputs["a_dst2"])], axis=1),
         np.asarray(inputs["b1"]).reshape(D, 1)],
        axis=1).astype(np.float32)
    base = {
        "x_in": x,
        "params_in": np.ascontiguousarray(params),
        "b2row_in": np.asarray(inputs["b2"], dtype=np.float32).reshape(1, D),
        "uidx_in": l1["uidx16"],
        "l1_eidx_in": l1["l1_eidx"],
        "l1_f_in": l1["l1_f"],
    }
    in_maps = []
    for c in range(NCORES):
        m = dict(base)
        m["eidx2_in"] = cores[c]["eidx2"]
        m["l2f_in"] = cores[c]["l2f"]
        in_maps.append(m)
    return in_maps


def unshard_core(oc, order, ncompb):
    got = np.empty((NPC, D), np.float32)
    nposc = ncompb * P
    pos = np.arange(nposc)
    got[order[:nposc]] = oc[(pos % P) * ncompb + pos // P]
    got[order[nposc:NPC]] = oc[nposc:NPC]
    return got


def unshard(results, cores, meta):
    out = np.empty((N, D), np.float32)
    for c in range(NCORES):
        out[c * NPC:(c + 1) * NPC] = unshard_core(
            results[c]["out"], cores[c]["order"], meta["ncompb"])
    return out


def kernel(**inputs):
    meta, l1, cores = prep(inputs)
    nc = build(meta, repeat=1)
    in_maps = make_in_maps(inputs, meta, l1, cores)
    res = run_bass_kernel_spmd(nc, in_maps, core_ids=list(range(NCORES)))
    return unshard(res.results, cores, meta)


# revision 35
# speedup vs baseline: 3.4714x; 1.4056x over previous
"""GAT 2-layer encoder on 8 Trainium2 NeuronCores.

Reference computation: layer 1 = GAT conv over edge_index[:, :500] (weights W1),
layer 2 = GAT conv over edge_index[:, 500:] (weights W2).

Strategy (sparse-special):
  - Layer-1 output x1 differs from b1 only on the K<=500 distinct dsts of the
    first 500 edges ("specials").  In layer 2, h2[src] = x1[src]@W2 is the
    constant default row for every non-special src, so only edges whose src is
    special (~8k of 1.6M) carry information.  For a dst with no special
    in-edge, softmax over equal scores gives alpha = 1/deg for every in-edge,
    hence out = b1@W2 + b2 exactly (up to the 1e-16 eps), a CONSTANT row.
  - Device builds a (K+2)-row table in h2-space with b2 baked in:
    row r = [x1_r@W2 + b2 | asrc2_r | adst2_r], row 0 = default, row K+1 = b2
    (for deg-0 dsts), rows K+2.. = replicas of row 0 (spread gather load).
  - Sharding: dst-range partition of the 1.6M layer-2 edges across 8 cores (no
    collectives; layer 1 + table build replicated on every core, it is tiny).
  - Per core: dsts sorted so that special-adst / deg-0 dsts land in block 0
    (full slot grid: special slots + default slot + dst slot, all gathered),
    remaining computed blocks carry ONLY special-edge slots; their default
    in-edge mass and adst2 = c_d are handled with per-position scalars.  The
    ~90 all-default blocks are written with one broadcast DMA of the constant
    row.  Layer-2 table rows are pre-transformed by W2, so no matmul there.
"""

import sys

sys.path.insert(0, "/opt/trn_rl_repo")

from contextlib import ExitStack

import numpy as np

import concourse.bacc as bacc
import concourse.bass as bass
import concourse.mybir as mybir
import concourse.tile as tile
from concourse.bass_utils import run_bass_kernel_spmd
from concourse.masks import make_identity

F32 = mybir.dt.float32
I16 = mybir.dt.int16
I32 = mybir.dt.int32
AF = mybir.ActivationFunctionType
OP = mybir.AluOpType

N = 100000
D = 64
NCORES = 8
NPC = N // NCORES          # dst nodes per core
P = 128
NSPLIT = 500               # first 500 edges -> layer 1
NEG_SLOPE = 0.2
BIG = 200.0                # score shift so padded slots underflow exp to 0.0
VTAB = 1024                # gather table rows (specials + default replicas)
NREPL = 512                # default-row replicas written (one broadcast DMA)


def _wrap16(flat):
    """int16 stream [n] (n%16==0) -> dma_gather idx tile [128, n//16]."""
    w = flat.reshape(-1, 16).T
    return np.ascontiguousarray(np.tile(w, (8, 1)).astype(np.int16))


def _groups_of(Ls, b0=0):
    """Split the per-block padded-degree profile into equal-L runs."""
    groups = []
    off = 0
    b = 0
    while b < len(Ls):
        s = b
        while b < len(Ls) and Ls[b] == Ls[s]:
            b += 1
        groups.append({"b0": b0 + s, "B": b - s, "L": Ls[s], "slot_off": off})
        off += (b - s) * Ls[s]
    return groups


def prep(inputs):
    """Host-side index prep (pure index computation, no feature values)."""
    ei = np.asarray(inputs["edge_index"])
    src = ei[0].astype(np.int64)
    dst = ei[1].astype(np.int64)
    s1, d1 = src[:NSPLIT], dst[:NSPLIT]
    s2, d2 = src[NSPLIT:], dst[NSPLIT:]

    # ---- layer 1 structure ----
    specials, deg1 = np.unique(d1, return_counts=True)
    K = len(specials)
    order1 = np.argsort(-deg1, kind="stable")
    spec_by_pos = specials[order1]          # grid position q -> node, table row q+1
    rowmap = np.zeros(N, np.int16)
    rowmap[spec_by_pos] = np.arange(1, K + 1)
    nblk1 = (K + P - 1) // P
    npos1 = nblk1 * P

    U = np.unique(np.concatenate([s1, d1]))
    nU = len(U)
    # x-row gather in int16-addressable ranges of 32768 rows
    RSPAN = 1 << 15
    xranges = []          # (lo, ntiles)
    uidx16_parts = []
    uindex = np.zeros(N, np.int64)
    off = 0
    for lo in range(0, N, RSPAN):
        hi = min(lo + RSPAN, N)
        Ur = U[(U >= lo) & (U < hi)]
        if len(Ur) == 0:
            continue
        nt = (len(Ur) + P - 1) // P
        pad = np.full(nt * P, lo, np.int64)
        pad[:len(Ur)] = Ur
        uindex[Ur] = off * P + np.arange(len(Ur))
        uidx16_parts.append(_wrap16((pad - lo).astype(np.int16)))
        xranges.append((lo, nt))
        off += nt
    nUt = off
    uidx16 = np.concatenate(uidx16_parts, axis=1)

    # layer-1 slot grid: per block [special-edge slots | dst slot]
    rank1 = np.empty(K, np.int64)
    rank1[order1] = np.arange(K)
    d1pos = rank1[np.searchsorted(specials, d1)]
    deg1_sorted = np.zeros(npos1, np.int64)
    deg1_sorted[:K] = deg1[order1]
    L1sp = [max(int(deg1_sorted[b * P:(b + 1) * P].max()), 1)
            for b in range(nblk1)]
    L1 = [l + 1 for l in L1sp]
    S1 = int(sum(L1))
    slot_base1 = np.concatenate([[0], np.cumsum(L1)])[:-1]
    idx1 = np.zeros(S1 * P, np.int16)
    mask1 = np.zeros(S1 * P, np.float32)
    pe = np.argsort(d1pos, kind="stable")
    pos_s = d1pos[pe]
    val_s = uindex[s1[pe]].astype(np.int16)
    start_of_pos = np.searchsorted(pos_s, np.arange(npos1))
    kk = np.arange(len(pos_s)) - start_of_pos[pos_s]
    flat = (slot_base1[pos_s // P] + kk) * P + (pos_s % P)
    idx1[flat] = val_s
    mask1[flat] = 1.0
    # dst slots (last slot of each block)
    posn = np.arange(npos1)
    dv1 = np.zeros(npos1, np.int16)
    dv1[:K] = uindex[spec_by_pos]
    fdst = (slot_base1[posn // P] + np.asarray(L1)[posn // P] - 1) * P + posn % P
    idx1[fdst] = dv1
    dp1 = np.zeros(npos1, np.float32)
    dp1[:K] = 1.0
    l1_eidx = _wrap16(idx1)
    l1_f = np.concatenate(
        [np.ascontiguousarray(mask1.reshape(S1, P).T),
         np.ascontiguousarray(dp1.reshape(nblk1, P).T)], axis=1)
    groups1 = _groups_of(L1)

    # ---- layer 2 structure (sparse-special grid) ----
    npos = ((NPC + P - 1) // P) * P
    nblk2 = npos // P
    ROW_B2 = K + 1
    REPL_LO = K + 2
    REPL_HI = REPL_LO + NREPL
    assert REPL_HI <= VTAB
    percore = []
    for c in range(NCORES):
        sel = (d2 >= c * NPC) & (d2 < (c + 1) * NPC)
        dl = d2[sel] - c * NPC
        sl = s2[sel]
        deg = np.bincount(dl, minlength=NPC)
        spr_all = rowmap[sl]
        m = spr_all > 0
        spd = dl[m]
        spr = spr_all[m]
        deg_sp = np.bincount(spd, minlength=NPC)
        ndef = deg - deg_sp
        spadst = rowmap[c * NPC:(c + 1) * NPC] > 0
        front = spadst | (deg == 0)
        assert int(front.sum()) <= P
        key = front.astype(np.int64) * (1 << 20) + deg_sp
        order = np.argsort(-key, kind="stable")
        ncomp = int((key > 0).sum())
        percore.append(dict(deg=deg, deg_sp=deg_sp, ndef=ndef, spd=spd,
                            spr=spr, order=order, ncomp=ncomp))
    ncompb = max(1, max((pc["ncomp"] + P - 1) // P for pc in percore))
    assert ncompb * P <= NPC

    def blkmax(pc, b):
        return int(pc["deg_sp"][pc["order"][b * P:(b + 1) * P]].max())

    L0 = max(blkmax(pc, 0) for pc in percore) + 2   # +default +dst slot
    Lb = [max(max(blkmax(pc, b) for pc in percore), 1)
          for b in range(1, ncompb)]
    S_g = L0 + int(sum(Lb))
    slot_base = np.zeros(ncompb, np.int64)                 # per-block slot base
    if ncompb > 1:
        slot_base[1:] = L0 + np.concatenate([[0], np.cumsum(Lb)[:-1]])
    cap = np.asarray([L0 - 2] + Lb)                        # special capacity
    groups2b = _groups_of(Lb, b0=1)

    cores = []
    nposc = ncompb * P
    for c, pc in enumerate(percore):
        deg, deg_sp, ndef = pc["deg"], pc["deg_sp"], pc["ndef"]
        spd, spr, order = pc["spd"], pc["spr"], pc["order"]
        rng = np.random.default_rng(1000 + c)
        idxflat = rng.integers(REPL_LO, REPL_HI, S_g * P).astype(np.int16)
        maskflat = np.zeros(S_g * P, np.float32)
        wts0 = np.zeros(L0 * P, np.float32)
        rank = np.empty(NPC, np.int64)
        rank[order] = np.arange(NPC)
        # special-edge slots
        pos = rank[spd]
        pe = np.argsort(pos, kind="stable")
        pos_s = pos[pe]
        val_s = spr[pe].astype(np.int16)
        assert pos_s.size == 0 or pos_s.max() < nposc
        start_of_pos = np.searchsorted(pos_s, np.arange(nposc))
        kk = np.arange(len(pos_s)) - start_of_pos[pos_s]
        assert np.all(kk < cap[pos_s // P])
        flat = (slot_base[pos_s // P] + kk) * P + (pos_s % P)
        idxflat[flat] = val_s
        maskflat[flat] = 1.0
        # block 0: default + dst slots
        p0 = np.arange(P)
        nodes0 = order[p0]
        nd0 = ndef[nodes0].astype(np.float32)
        is00 = deg[nodes0] == 0
        fd = (L0 - 2) * P + p0
        ft = (L0 - 1) * P + p0
        maskflat[fd] = ((nd0 > 0) | is00).astype(np.float32)
        wts0[0:(L0 - 2) * P] = maskflat[0:(L0 - 2) * P]
        wts0[fd] = np.where(is00, 1.0, nd0)
        idxflat[fd] = np.where(is00, np.int16(ROW_B2), idxflat[fd])
        rm0 = rowmap[c * NPC + nodes0]
        h0 = rm0 > 0
        idxflat[ft[h0]] = rm0[h0]
        # blocks >= 1 scalar fields
        posn = np.arange(P, nposc)
        nodesb = order[posn]
        assert np.all(deg[nodesb] > 0)
        m0b = (ndef[nodesb] > 0).astype(np.float32)
        w0b = ndef[nodesb].astype(np.float32)
        m0b_t = np.ascontiguousarray(m0b.reshape(ncompb - 1, P).T) \
            if ncompb > 1 else np.zeros((P, 0), np.float32)
        w0b_t = np.ascontiguousarray(w0b.reshape(ncompb - 1, P).T) \
            if ncompb > 1 else np.zeros((P, 0), np.float32)
        l2f = np.concatenate(
            [np.ascontiguousarray(maskflat.reshape(S_g, P).T),
             np.ascontiguousarray(wts0.reshape(L0, P).T),
             m0b_t, w0b_t], axis=1)
        cores.append({"eidx2": _wrap16(idxflat), "l2f": l2f, "order": order})

    meta = {
        "K": K, "nblk1": nblk1, "nU": nU, "nUt": nUt, "xranges": xranges,
        "L1": L1, "groups1": groups1, "S1": S1,
        "L0": L0, "Lb": Lb, "S_g": S_g, "groups2b": groups2b,
        "ncompb": ncompb, "nblk2": nblk2, "npos": npos,
        "ROW_B2": ROW_B2, "REPL_LO": REPL_LO,
    }
    l1 = {"uidx16": uidx16, "l1_eidx": l1_eidx, "l1_f": l1_f}
    return meta, l1, cores


def _extract_lastslot(nc, gw, Gap, B, L, col, tag):
    """[P, B] tile holding Gap[:, b*L + L-1, col] per block b."""
    t = gw.tile([P, B], F32, tag=tag)
    nc.scalar.activation(
        t[:],
        Gap[:, :, col:col + 1]
        .rearrange("p (b l) o -> p b (l o)", l=L)[:, :, L - 1:L]
        .rearrange("p b o -> p (b o)"),
        AF.Identity)
    return t


def _emit_group(nc, gw, Gap, mask_ap, adst_ap, B, L, wts_ap=None,
                degpos_ap=None, out_ap=None):
    """Segment softmax + weighted sum for B blocks of equal padded degree L.

    Gap: AP view [128, B*L, 128] of the gathered rows (slot-flat).
    Returns msg tile [128, B, 64]."""
    BL = B * L
    asrc = Gap[:, :, 64:65].rearrange("p s o -> p (s o)")        # [128, BL]
    s_t = gw.tile([P, B, L], F32, tag="s_t")
    nc.vector.tensor_tensor(s_t[:], asrc, adst_ap.to_broadcast((P, B, L)),
                            op=OP.add)
    u_t = gw.tile([P, B, L], F32, tag="u_t")
    nc.vector.scalar_tensor_tensor(u_t[:], s_t[:], NEG_SLOPE, s_t[:],
                                   op0=OP.mult, op1=OP.max)
    e2_t = gw.tile([P, B, L], F32, tag="e2_t")
    nc.vector.scalar_tensor_tensor(e2_t[:], u_t[:], BIG, mask_ap,
                                   op0=OP.add, op1=OP.mult)
    mneg = gw.tile([P, B], F32, tag="mneg")
    nc.vector.tensor_reduce(mneg[:], e2_t[:], axis=mybir.AxisListType.X,
                            op=OP.max, negate=True)
    d_t = gw.tile([P, B, L], F32, tag="d_t")
    nc.vector.tensor_tensor(d_t[:], e2_t[:], mneg[:].to_broadcast((P, B, L)),
                            op=OP.add)
    ex_t = gw.tile([P, B, L], F32, tag="ex_t")
    nc.scalar.activation(ex_t[:], d_t[:], AF.Exp)
    if wts_ap is not None:
        exw_t = gw.tile([P, B, L], F32, tag="exw_t")
        nc.vector.tensor_tensor(exw_t[:], ex_t[:], wts_ap, op=OP.mult)
    else:
        exw_t = ex_t
    ssum = gw.tile([P, B], F32, tag="ssum")
    nc.vector.tensor_reduce(ssum[:], exw_t[:], axis=mybir.AxisListType.X,
                            op=OP.add)
    rs = gw.tile([P, B], F32, tag="rs")
    nc.vector.reciprocal(rs[:], ssum[:])
    if degpos_ap is not None:
        rsd = gw.tile([P, B], F32, tag="rsd")
        nc.vector.tensor_tensor(rsd[:], rs[:], degpos_ap, op=OP.mult)
    else:
        rsd = rs
    alpha = gw.tile([P, B, L], F32, tag="alpha")
    nc.vector.tensor_tensor(alpha[:], exw_t[:], rsd[:].to_broadcast((P, B, L)),
                            op=OP.mult)
    wr = gw.tile([P, BL, D], F32, tag="wr")
    nc.vector.tensor_tensor(wr[:], Gap[:, :, 0:D],
                            alpha[:].rearrange("p b l -> p (b l)")
                            .to_broadcast((P, BL, D)), op=OP.mult)
    if out_ap is None:
        msg = gw.tile([P, B, D], F32, tag="msg")
        out_ap = msg[:]
    else:
        msg = None
    nc.vector.tensor_reduce(out_ap,
                            wr[:].rearrange("p (b l) f -> p b f l", b=B),
                            axis=mybir.AxisListType.X, op=OP.add)
    return msg


def _emit_sp_group(nc, gw, Gap, mask_ap, m0b_ap, w0b_ap, e0c, cdrep, defrowv,
                   B, L, out_ap=None):
    """Blocks with only special-edge slots: default-edge mass via scalars.

    Gap [128, B*L, 128]; adst = c_d (non-special dsts); e0c [P,1] = default
    score lrelu(c_s+c_d)+BIG; m0b/w0b [P,B] = (ndef>0) and ndef.
    Returns msg tile [128, B, 64] (includes the default-row contribution)."""
    BL = B * L
    asrc = Gap[:, :, 64:65].rearrange("p s o -> p (s o)")
    s_t = gw.tile([P, B, L], F32, tag="sp_s")
    nc.vector.tensor_tensor(s_t[:], asrc, cdrep.to_broadcast((P, B, L)),
                            op=OP.add)
    u_t = gw.tile([P, B, L], F32, tag="sp_u")
    nc.vector.scalar_tensor_tensor(u_t[:], s_t[:], NEG_SLOPE, s_t[:],
                                   op0=OP.mult, op1=OP.max)
    e2_t = gw.tile([P, B, L], F32, tag="sp_e2")
    nc.vector.scalar_tensor_tensor(e2_t[:], u_t[:], BIG, mask_ap,
                                   op0=OP.add, op1=OP.mult)
    e0e = gw.tile([P, B], F32, tag="sp_e0")
    nc.vector.tensor_tensor(e0e[:], m0b_ap, e0c.to_broadcast((P, B)),
                            op=OP.mult)
    m_t = gw.tile([P, B], F32, tag="sp_m")
    if L > 1:
        msp = gw.tile([P, B], F32, tag="sp_msp")
        nc.vector.tensor_reduce(msp[:], e2_t[:], axis=mybir.AxisListType.X,
                                op=OP.max)
        nc.vector.tensor_tensor(m_t[:], msp[:], e0e[:], op=OP.max)
    else:
        nc.vector.tensor_tensor(
            m_t[:], e2_t[:].rearrange("p b l -> p (b l)"), e0e[:], op=OP.max)
    d_t = gw.tile([P, B, L], F32, tag="sp_d")
    nc.vector.tensor_tensor(d_t[:], e2_t[:], m_t[:].to_broadcast((P, B, L)),
                            op=OP.subtract)
    ex_t = gw.tile([P, B, L], F32, tag="sp_ex")
    nc.scalar.activation(ex_t[:], d_t[:], AF.Exp)
    d0 = gw.tile([P, B], F32, tag="sp_d0")
    nc.vector.tensor_tensor(d0[:], e0e[:], m_t[:], op=OP.subtract)
    ex0 = gw.tile([P, B], F32, tag="sp_ex0")
    nc.scalar.activation(ex0[:], d0[:], AF.Exp)
    exw0 = gw.tile([P, B], F32, tag="sp_exw0")
    nc.vector.tensor_tensor(exw0[:], ex0[:], w0b_ap, op=OP.mult)
    ssum = gw.tile([P, B], F32, tag="sp_ssum")
    if L > 1:
        ssp = gw.tile([P, B], F32, tag="sp_ssp")
        nc.vector.tensor_reduce(ssp[:], ex_t[:], axis=mybir.AxisListType.X,
                                op=OP.add)
        nc.vector.tensor_tensor(ssum[:], ssp[:], exw0[:], op=OP.add)
    else:
        nc.vector.tensor_tensor(
            ssum[:], ex_t[:].rearrange("p b l -> p (b l)"), exw0[:], op=OP.add)
    rs = gw.tile([P, B], F32, tag="sp_rs")
    nc.vector.reciprocal(rs[:], ssum[:])
    alpha = gw.tile([P, B, L], F32, tag="sp_al")
    nc.vector.tensor_tensor(alpha[:], ex_t[:], rs[:].to_broadcast((P, B, L)),
                            op=OP.mult)
    alpha0 = gw.tile([P, B], F32, tag="sp_al0")
    nc.vector.tensor_tensor(alpha0[:], exw0[:], rs[:], op=OP.mult)
    wr = gw.tile([P, BL, D], F32, tag="sp_wr")
    nc.vector.tensor_tensor(wr[:], Gap[:, :, 0:D],
                            alpha[:].rearrange("p b l -> p (b l)")
                            .to_broadcast((P, BL, D)), op=OP.mult)
    if L > 1:
        msgs = gw.tile([P, B, D], F32, tag="sp_msgs")
        nc.vector.tensor_reduce(
            msgs[:], wr[:].rearrange("p (b l) f -> p b f l", b=B),
            axis=mybir.AxisListType.X, op=OP.add)
    else:
        msgs = wr
    t1 = gw.tile([P, B, D], F32, tag="sp_t1")
    nc.vector.tensor_tensor(t1[:], alpha0[:].to_broadcast((P, B, D)),
                            defrowv.to_broadcast((P, B, D)), op=OP.mult)
    if out_ap is None:
        msg = gw.tile([P, B, D], F32, tag="sp_msg")
        out_ap = msg[:]
    else:
        msg = None
    nc.vector.tensor_tensor(out_ap, msgs[:], t1[:], op=OP.add)
    return msg


def build(meta, repeat=1, stages="ducge"):
    """Build the SPMD Bass program (common across cores).

    stages: subset of 'd' (default writes), 'u' (l1 endpoint table),
    'c' (l1 conv -> tab rows), 'g' (l2 gather), 'e' (l2 emit+write)."""
    K = meta["K"]
    nblk1, nUt = meta["nblk1"], meta["nUt"]
    S1, groups1, L1 = meta["S1"], meta["groups1"], meta["L1"]
    S_g, L0, groups2b = meta["S_g"], meta["L0"], meta["groups2b"]
    ncompb, nblk2 = meta["ncompb"], meta["nblk2"]
    ROW_B2, REPL_LO = meta["ROW_B2"], meta["REPL_LO"]
    NPRM = 4 * D + 5
    nf1 = S1 + nblk1
    nf2 = S_g + L0 + 2 * (ncompb - 1)

    nc = bacc.Bacc("TRN2", target_bir_lowering=False, debug=False,
                   num_devices=NCORES)
    dt = nc.dram_tensor
    xranges = meta["xranges"]
    x_in = dt("x_in", [N, D], F32, kind="ExternalInput").ap()
    params_in = dt("params_in", [D, NPRM], F32, kind="ExternalInput").ap()
    b2row_in = dt("b2row_in", [1, D], F32, kind="ExternalInput").ap()
    uidx_in = dt("uidx_in", [P, 8 * nUt], I16, kind="ExternalInput").ap()
    l1_eidx_in = dt("l1_eidx_in", [P, 8 * S1], I16, kind="ExternalInput").ap()
    l1_f_in = dt("l1_f_in", [P, nf1], F32, kind="ExternalInput").ap()
    eidx2_in = dt("eidx2_in", [P, 8 * S_g], I16, kind="ExternalInput").ap()
    l2f_in = dt("l2f_in", [P, nf2], F32, kind="ExternalInput").ap()
    out_t = dt("out", [meta["npos"], D], F32, kind="ExternalOutput").ap()

    h1tab = dt("h1tab", [nUt * P, P], F32).ap()
    tab = dt("tab", [VTAB, P], F32).ap()

    with tile.TileContext(nc) as tc, ExitStack() as ctx:
        const = ctx.enter_context(tc.tile_pool(name="const", bufs=1))
        psc_ctx = tc.tile_pool(name="psc", bufs=1, space="PSUM")
        psc = psc_ctx.__enter__()

        ident = const.tile([P, P], F32)
        make_identity(nc, ident[:])

        # ---- parameters (one blob: W1|W1T|W2|W2T|av1|av2|b1col) ----
        prm = const.tile([D, NPRM], F32)
        nc.sync.dma_start(prm[:], params_in[:])
        W1s = prm[:, 0:D]
        W1Ts = prm[:, D:2 * D]
        W2s = prm[:, 2 * D:3 * D]
        W2Ts = prm[:, 3 * D:4 * D]
        av1s = prm[:, 4 * D:4 * D + 2]
        av2s = prm[:, 4 * D + 2:4 * D + 4]
        b1cols = prm[:, 4 * D + 4:4 * D + 5]
        b2rows = const.tile([1, D], F32)
        nc.sync.dma_start(b2rows[:], b2row_in[:])

        wt1_p = psc.tile([D, 2], F32, space="PSUM")
        nc.tensor.matmul(wt1_p[:], W1Ts, av1s, start=True, stop=True)
        wt2_p = psc.tile([D, 2], F32, space="PSUM")
        nc.tensor.matmul(wt2_p[:], W2Ts, av2s, start=True, stop=True)
        wt2s = const.tile([D, 2], F32)
        nc.vector.tensor_copy(wt2s[:], wt2_p[:])

        W1aug = const.tile([D, D + 2], F32)
        nc.vector.tensor_copy(W1aug[:, 0:D], W1s)
        nc.vector.tensor_copy(W1aug[:, D:D + 2], wt1_p[:])

        # SPEC2 [65, 66] = [[W2 | wt2s wt2d]; [b1@W2+b2 | b1.wt2s b1.wt2d]]
        SPEC = const.tile([D + 1, D + 2], F32)
        nc.vector.tensor_copy(SPEC[0:D, 0:D], W2s)
        nc.vector.tensor_copy(SPEC[0:D, D:D + 2], wt2s[:])
        b1w2_p = psc.tile([1, D], F32, space="PSUM")
        nc.tensor.matmul(b1w2_p[:], b1cols, W2s, start=True, stop=True)
        nc.vector.tensor_tensor(SPEC[D:D + 1, 0:D], b1w2_p[:], b2rows[:],
                                op=OP.add)
        b1w_p = psc.tile([1, 2], F32, space="PSUM")
        nc.tensor.matmul(b1w_p[:], b1cols, wt2s[:], start=True, stop=True)
        nc.vector.tensor_copy(SPEC[D:D + 1, D:D + 2], b1w_p[:])

        # block-diagonal pair matrices (two 64-wide tiles per PE pass)
        W1aug2 = const.tile([2 * D, 2 * (D + 2)], F32)
        nc.vector.memset(W1aug2[:], 0.0)
        nc.vector.tensor_copy(W1aug2[0:D, 0:D + 2], W1aug[:])
        nc.vector.tensor_copy(W1aug2[D:2 * D, D + 2:2 * (D + 2)], W1aug[:])
        W2D = const.tile([2 * D, 2 * (D + 2)], F32)
        nc.vector.memset(W2D[:], 0.0)
        nc.vector.tensor_copy(W2D[0:D, 0:D + 2], SPEC[0:D, :])
        nc.vector.tensor_copy(W2D[D:2 * D, D + 2:2 * (D + 2)], SPEC[0:D, :])

        # ---- constant table rows + replicated default row ----
        row0_s = const.tile([1, P], F32)
        nc.vector.memset(row0_s[:], 0.0)
        nc.vector.tensor_copy(row0_s[:, 0:D + 2], SPEC[D:D + 1, :])
        b2r_s = const.tile([1, P], F32)
        nc.vector.memset(b2r_s[:], 0.0)
        nc.vector.tensor_copy(b2r_s[:, 0:D], b2rows[:])
        ones_s = const.tile([1, P], F32)
        nc.vector.memset(ones_s[:], 1.0)
        repl_p = psc.tile([P, P], F32, space="PSUM")
        nc.tensor.matmul(repl_p[:], ones_s[:], row0_s[:], start=True, stop=True)
        repl_s = const.tile([P, P], F32)
        nc.vector.tensor_copy(repl_s[:], repl_p[:])
        defrowv = repl_s[:, 0:D].rearrange("p (k f) -> p k f", k=1)
        csrep = repl_s[:, D:D + 1]
        cdrep = repl_s[:, D + 1:D + 2]
        replv = repl_s[:].rearrange("p (k f) -> p k f", k=1)
        KB = 16                      # blocks per default-write chunk
        defbig = const.tile([P, KB * D], F32)
        nc.vector.tensor_copy(
            defbig[:].rearrange("p (k f) -> p k f", k=KB),
            defrowv.to_broadcast((P, KB, D)))

        nc.sync.dma_start(tab[0:1, :], row0_s[:])
        nc.sync.dma_start(tab[ROW_B2:ROW_B2 + 1, :], b2r_s[:])
        nc.sync.dma_start(
            tab[REPL_LO:REPL_LO + NREPL, :].rearrange("(k p) f -> p k f", p=P),
            replv.to_broadcast((P, NREPL // P, P)))

        # default-score constant e0c = lrelu(c_s + c_d) + BIG
        s0c = const.tile([P, 1], F32)
        nc.vector.tensor_tensor(s0c[:], csrep, cdrep, op=OP.add)
        u0c = const.tile([P, 1], F32)
        nc.vector.scalar_tensor_tensor(u0c[:], s0c[:], NEG_SLOPE, s0c[:],
                                       op0=OP.mult, op1=OP.max)
        e0c = const.tile([P, 1], F32)
        nc.vector.tensor_scalar_add(e0c[:], u0c[:], BIG)

        psc_ctx.__exit__(None, None, None)

        # ---- index tensors ----
        uidx_s = const.tile([P, 8 * nUt], I16)
        nc.sync.dma_start(uidx_s[:], uidx_in[:])
        l1_eidx_s = const.tile([P, 8 * S1], I16)
        nc.sync.dma_start(l1_eidx_s[:], l1_eidx_in[:])
        l1_f_s = const.tile([P, nf1], F32)
        nc.sync.dma_start(l1_f_s[:], l1_f_in[:])
        l1_mask_s = l1_f_s[:, 0:S1]
        l1_degpos_s = l1_f_s[:, S1:S1 + nblk1]
        eidx2_s = const.tile([P, 8 * S_g], I16)
        nc.sync.dma_start(eidx2_s[:], eidx2_in[:])
        l2f_s = const.tile([P, nf2], F32)
        nc.sync.dma_start(l2f_s[:], l2f_in[:])
        mask2_s = l2f_s[:, 0:S_g]
        wts0_s = l2f_s[:, S_g:S_g + L0]
        m0b_s = l2f_s[:, S_g + L0:S_g + L0 + (ncompb - 1)]
        w0b_s = l2f_s[:, S_g + L0 + (ncompb - 1):nf2]

        repl2 = const.tile([P, 2 * (D + 2)], F32)
        nc.vector.tensor_copy(repl2[:, 0:D + 2], repl_s[:, 0:D + 2])
        nc.vector.tensor_copy(repl2[:, D + 2:2 * (D + 2)],
                              repl_s[:, 0:D + 2])
        npr1 = nblk1 // 2

        ndefblk = nblk2 - ncompb
        outcmp = out_t[0:ncompb * P, :].rearrange("(p b) f -> p b f", b=ncompb)

        l1u = ctx.enter_context(tc.tile_pool(name="l1u", bufs=3))
        l1up = ctx.enter_context(tc.tile_pool(name="l1up", bufs=2,
                                              space="PSUM"))
        l1w = ctx.enter_context(tc.tile_pool(name="l1w", bufs=3))
        l1p = ctx.enter_context(tc.tile_pool(name="l1p", bufs=2, space="PSUM"))
        gw = ctx.enter_context(tc.tile_pool(name="gw", bufs=3))

        for _rep in range(repeat):
            # ---- default-region output: chunked contiguous writes ----
            if "d" in stages:
                b = 0
                while b < ndefblk:
                    nb = min(KB, ndefblk - b)
                    r0 = (ncompb + b) * P
                    nc.sync.dma_start(
                        out_t[r0:r0 + nb * P, :].rearrange(
                            "(p k) f -> p (k f)", k=nb),
                        defbig[:, 0:nb * D])
                    b += nb

            # ---- layer 1: h1 table for the U endpoint nodes ----
            if "u" not in stages:
                continue
            if True:
                xall = l1u.tile([P, nUt, D], F32, tag="xall")
                h_all = l1u.tile([P, nUt, D + 2], F32, tag="h_all")
                toff = 0
                for lo, nt in xranges:
                    hi = min(lo + (1 << 15), N)
                    nc.gpsimd.dma_gather(
                        xall[:, toff:toff + nt, :], x_in[lo:hi, :],
                        uidx_s[:, 8 * toff:8 * (toff + nt)],
                        nt * P, nt * P, D, single_packet=False)
                    toff += nt
                for t2 in range(nUt // 2):
                    xT_p = l1up.tile([P, P], F32, space="PSUM", tag="xT")
                    nc.tensor.transpose(
                        xT_p[:],
                        xall[:, 2 * t2:2 * t2 + 2, :]
                        .rearrange("p k f -> p (k f)"), ident[:])
                    xT_s = l1u.tile([P, P], F32, tag="xTs")
                    nc.vector.tensor_copy(xT_s[:], xT_p[:])
                    h_p = l1up.tile([P, 2 * (D + 2)], F32, space="PSUM",
                                    tag="h_p")
                    nc.tensor.matmul(h_p[:], xT_s[:], W1aug2[:], start=True,
                                     stop=True)
                    nc.scalar.copy(
                        h_all[:, 2 * t2:2 * t2 + 2, :]
                        .rearrange("p k f -> p (k f)"), h_p[:])
                if nUt % 2:
                    t = nUt - 1
                    xT_p1 = l1up.tile([D, P], F32, space="PSUM", tag="xT1")
                    nc.tensor.transpose(xT_p1[:], xall[:, t, :], ident[:])
                    xT_s1 = l1u.tile([D, P], F32, tag="xTs1")
                    nc.vector.tensor_copy(xT_s1[:], xT_p1[:])
                    h_p1 = l1up.tile([P, D + 2], F32, space="PSUM", tag="h_p1")
                    nc.tensor.matmul(h_p1[:], xT_s1[:], W1aug[:], start=True,
                                     stop=True)
                    nc.scalar.copy(h_all[:, t, :], h_p1[:])
                nc.sync.dma_start(
                    h1tab[:, 0:D + 2].rearrange("(k p) f -> p k f", p=P),
                    h_all[:])

            # ---- layer 1 conv -> write special table rows 1..K ----
            if "c" not in stages:
                continue
            if True:
                G1 = l1w.tile([P, S1, P], F32, tag="G1")
                mTs_c = l1w.tile([2 * D, max(npr1, 1), P], F32, tag="mTs_c")
                mTs_s = l1w.tile([D, P], F32, tag="mTs_s")
                row_all = l1w.tile([P, nblk1, D + 2], F32, tag="row_all")
                msg1_all = l1w.tile([P, nblk1, D], F32, tag="msg1_all")
                nc.gpsimd.dma_gather(G1[:], h1tab[:, :], l1_eidx_s[:],
                                     S1 * P, S1 * P, P, single_packet=False)
                for g in groups1:
                    B, L, off = g["B"], g["L"], g["slot_off"]
                    Gap = G1[:, off:off + B * L, :]
                    adst1 = _extract_lastslot(nc, l1w, Gap, B, L, D + 1,
                                              "adst1")
                    _emit_group(
                        nc, l1w, Gap, l1_mask_s[:, off:off + B * L],
                        adst1[:], B, L,
                        degpos_ap=l1_degpos_s[:, g["b0"]:g["b0"] + B],
                        out_ap=msg1_all[:, g["b0"]:g["b0"] + B, :])
                for pr in range(nblk1 // 2):
                    mT_p = l1p.tile([P, P], F32, space="PSUM", tag="mT")
                    nc.tensor.transpose(
                        mT_p[:],
                        msg1_all[:, 2 * pr:2 * pr + 2, :]
                        .rearrange("p k f -> p (k f)"), ident[:])
                    nc.vector.tensor_copy(mTs_c[:, pr, :], mT_p[:])
                    row_p = l1p.tile([P, 2 * (D + 2)], F32, space="PSUM",
                                     tag="rowp")
                    nc.tensor.matmul(row_p[:], mTs_c[:, pr, :], W2D[:],
                                     start=True, stop=True)
                    nc.vector.tensor_tensor(
                        row_all[:, 2 * pr:2 * pr + 2, :]
                        .rearrange("p k f -> p (k f)"), row_p[:], repl2[:],
                        op=OP.add)
                if nblk1 % 2:
                    b = nblk1 - 1
                    mT_p1 = l1p.tile([D, P], F32, space="PSUM", tag="mT1")
                    nc.tensor.transpose(mT_p1[:], msg1_all[:, b, :], ident[:])
                    nc.vector.tensor_copy(mTs_s[:], mT_p1[:])
                    row_p1 = l1p.tile([P, D + 2], F32, space="PSUM",
                                      tag="rowp1")
                    nc.tensor.matmul(row_p1[:], mTs_s[:], SPEC[0:D, :],
                                     start=True, stop=True)
                    nc.vector.tensor_tensor(row_all[:, b, :], row_p1[:],
                                            repl2[:, 0:D + 2], op=OP.add)
                nfull = K // P
                if nfull:
                    nc.sync.dma_start(
                        tab[1:1 + nfull * P, 0:D + 2].rearrange(
                            "(k p) f -> p k f", p=P),
                        row_all[:, 0:nfull, :])
                rem = K - nfull * P
                if rem:
                    nc.sync.dma_start(tab[1 + nfull * P:1 + K, 0:D + 2],
                                      row_all[0:rem, nfull, :])

            # ---- layer 2 ----
            if "g" not in stages:
                continue
            if True:
                G = gw.tile([P, S_g, P], F32, tag="G")
                msgall = gw.tile([P, ncompb, D], F32, tag="msgall")
                nc.gpsimd.dma_gather(G[:], tab[:, :], eidx2_s[:],
                                     S_g * P, S_g * P, P, single_packet=False)
                if "e" not in stages:
                    dum = gw.tile([P, P], F32, tag="dum")
                    nc.vector.tensor_copy(dum[:], G[:, 0, :])
                    continue
                # block 0: full grid
                Gap0 = G[:, 0:L0, :]
                adst0 = _extract_lastslot(nc, gw, Gap0, 1, L0, D + 1, "adst0")
                _emit_group(nc, gw, Gap0, mask2_s[:, 0:L0], adst0[:],
                            1, L0, wts_ap=wts0_s[:],
                            out_ap=msgall[:, 0:1, :])
                # blocks >= 1: special-only grids
                for g in groups2b:
                    B, L, off = g["B"], g["L"], g["slot_off"]
                    Gap = G[:, L0 + off:L0 + off + B * L, :]
                    _emit_sp_group(
                        nc, gw, Gap, mask2_s[:, L0 + off:L0 + off + B * L],
                        m0b_s[:, g["b0"] - 1:g["b0"] - 1 + B],
                        w0b_s[:, g["b0"] - 1:g["b0"] - 1 + B],
                        e0c[:], cdrep, defrowv, B, L,
                        out_ap=msgall[:, g["b0"]:g["b0"] + B, :])
                nc.sync.dma_start(outcmp, msgall[:])

    nc.compile()
    return nc


def make_in_maps(inputs, meta, l1, cores):
    x = np.ascontiguousarray(np.asarray(inputs["x"], dtype=np.float32))
    W1 = np.asarray(inputs["W1"], dtype=np.float32)
    W2 = np.asarray(inputs["W2"], dtype=np.float32)
    params = np.concatenate(
        [W1, np.ascontiguousarray(W1.T), W2, np.ascontiguousarray(W2.T),
         np.stack([np.asarray(inputs["a_src1"]),
                   np.asarray(inputs["a_dst1"])], axis=1),
         np.stack([np.asarray(inputs["a_src2"]),
                   np.asarray(inputs["a_dst2"])], axis=1),
         np.asarray(inputs["b1"]).reshape(D, 1)],
        axis=1).astype(np.float32)
    base = {
        "x_in": x,
        "params_in": np.ascontiguousarray(params),
        "b2row_in": np.asarray(inputs["b2"], dtype=np.float32).reshape(1, D),
        "uidx_in": l1["uidx16"],
        "l1_eidx_in": l1["l1_eidx"],
        "l1_f_in": l1["l1_f"],
    }
    in_maps = []
    for c in range(NCORES):
        m = dict(base)
        m["eidx2_in"] = cores[c]["eidx2"]
        m["l2f_in"] = cores[c]["l2f"]
        in_maps.append(m)
    return in_maps


def unshard_core(oc, order, ncompb):
    got = np.empty((NPC, D), np.float32)
    nposc = ncompb * P
    pos = np.arange(nposc)
    got[order[:nposc]] = oc[(pos % P) * ncompb + pos // P]
    got[order[nposc:NPC]] = oc[nposc:NPC]
    return got


def unshard(results, cores, meta):
    out = np.empty((N, D), np.float32)
    for c in range(NCORES):
        out[c * NPC:(c + 1) * NPC] = unshard_core(
            results[c]["out"], cores[c]["order"], meta["ncompb"])
    return out


def kernel(**inputs):
    meta, l1, cores = prep(inputs)
    nc = build(meta, repeat=1)
    in_maps = make_in_maps(inputs, meta, l1, cores)
    res = run_bass_kernel_spmd(nc, in_maps, core_ids=list(range(NCORES)))
    return unshard(res.results, cores, meta)
